# revision 11
# baseline (speedup 1.0000x reference)
"""Trainium2 8-core fused attention kernel (QKV proj + RMSNorm + RoPE + SDPA + out proj).

Sharding: tensor-parallel over heads. Each of the 8 cores computes 2 of the 16
heads end-to-end (QKV projection with its Wqkv column shard, per-head RMSNorm +
RoPE, full softmax attention), then an AllToAll redistributes the per-head
attention outputs so every core holds all 1024 attention channels for 1/8 of
the tokens and applies the full Wout to its token shard.

Self-contained: hardcodes all shapes from the problem spec.
"""
import os
import sys
import types

import numpy as np

sys.path.insert(0, "/opt/trn_rl_repo")

from concourse import bass, bacc, tile, mybir  # noqa: E402
from concourse.bass_utils import run_bass_kernel_spmd  # noqa: E402
from concourse.masks import make_identity  # noqa: E402

B, N, C, H, D = 2, 4096, 1024, 16, 64
NCORES = 8
TOK = B * N            # 8192 global tokens
NB = N // 128          # 32 token tiles per batch
NMACRO = N // 256      # 16 macro tiles (256 tok) per batch
QTILE = 512
NQT = N // QTILE       # 8 q tiles per batch
KC = N // 128          # 32 key chunks per batch
SHARD = TOK // NCORES  # 1024 tokens per core shard
EPS = 1e-6

F32 = mybir.dt.float32
F32R = mybir.dt.float32r
BF16 = mybir.dt.bfloat16
ALU = mybir.AluOpType
ACTF = mybir.ActivationFunctionType

_CACHE = {}
_LAST_RESULT = None


def _install_profile_shim():
    """trn_boot skips the NTFF hook when antenv.axon_hooks is missing; supply it."""
    try:
        import antenv
        if getattr(antenv, "axon_hooks", None) is not None:
            return
        from trn_agent_boot.trn_boot import _ntff_profile_via_ctypes
        hook = _ntff_profile_via_ctypes("/opt/axon/libaxon_pjrt.so")
        if hook is None:
            return
        mod = types.ModuleType("antenv.axon_hooks")
        state = {"hook": hook}
        mod.get_axon_ntff_profile_hook = lambda: state["hook"]
        mod.set_axon_ntff_profile_hook = lambda h: state.__setitem__("hook", h)
        sys.modules["antenv.axon_hooks"] = mod
        antenv.axon_hooks = mod
    except Exception:
        pass


def _build_graph():
    nc = bacc.Bacc("TRN2", target_bir_lowering=False, debug=False,
                   enable_asserts=True, num_devices=NCORES)

    hsT_d = nc.dram_tensor("hsT", [C, TOK], F32, kind="ExternalInput")
    wqkv_d = nc.dram_tensor("wqkv", [C, 384], F32, kind="ExternalInput")
    cosq_d = nc.dram_tensor("cosq", [N, D], F32, kind="ExternalInput")
    sinq_d = nc.dram_tensor("sinq", [N, D], F32, kind="ExternalInput")
    cosk_d = nc.dram_tensor("cosk", [N, D], F32, kind="ExternalInput")
    sink_d = nc.dram_tensor("sink", [N, D], F32, kind="ExternalInput")
    wout_d = nc.dram_tensor("wout", [C, C], F32, kind="ExternalInput")
    out_d = nc.dram_tensor("out", [SHARD, C], F32, kind="ExternalOutput")

    with tile.TileContext(nc) as tc:
        with tc.tile_pool(name="const", bufs=1) as constp, \
             tc.tile_pool(name="dram", bufs=1, space="DRAM") as dram:
            # resident weights
            wqkv_sb = constp.tile([128, 8, 384], F32R)
            wout_sb = constp.tile([128, 8, C], BF16)
            with tc.tile_pool(name="wtmp", bufs=2) as wtmp:
                for cc in range(8):
                    wqt = wtmp.tile([128, 384], F32, name="wqt", tag="wqt")
                    nc.sync.dma_start(
                        wqt[:], wqkv_d.ap()[cc * 128:(cc + 1) * 128, :])
                    nc.vector.tensor_copy(wqkv_sb[:, cc, :], wqt[:])
                    wtt = wtmp.tile([128, C], F32, name="wtt", tag="wtt")
                    nc.sync.dma_start(
                        wtt[:], wout_d.ap()[cc * 128:(cc + 1) * 128, :])
                    nc.vector.tensor_copy(wout_sb[:, cc, :], wtt[:])
            ident = constp.tile([128, 128], F32)
            make_identity(nc, ident[:])
            ones_f = constp.tile([65, 64], F32)
            nc.vector.memset(ones_f[:], 1.0)
            ones_sb = constp.tile([65, 64], F32R)
            nc.vector.tensor_copy(ones_sb[:], ones_f[:])
            eps_sb = constp.tile([128, 1], F32)
            nc.vector.memset(eps_sb[:], EPS)
            neg3_sb = constp.tile([128, 1], F32)
            nc.vector.memset(neg3_sb[:], -3.0)

            a2a_in = dram.tile([NCORES, 128, SHARD], BF16)
            a2a_out = dram.tile([NCORES, 128, SHARD], BF16)

            with tc.tile_pool(name="batch", bufs=1) as bp, \
                 tc.tile_pool(name="stream", bufs=2) as sp, \
                 tc.tile_pool(name="work", bufs=3) as wp, \
                 tc.tile_pool(name="probsp", bufs=4) as pp, \
                 tc.tile_pool(name="ps1", bufs=2, space="PSUM") as ps1, \
                 tc.tile_pool(name="pssc", bufs=2, space="PSUM") as pssc, \
                 tc.tile_pool(name="psat", bufs=3, space="PSUM") as psat, \
                 tc.tile_pool(name="psbc", bufs=1, space="PSUM") as psbc:

                qT = [bp.tile([128, N], F32R, name=f"qT{b}", tag=f"qT{b}") for b in range(B)]
                kT = [bp.tile([128, N], F32R, name=f"kT{b}", tag=f"kT{b}") for b in range(B)]
                vsb = [[bp.tile([128, NB, 65], BF16, name=f"v{b}{h}", tag=f"v{b}{h}")
                        for h in range(2)] for b in range(B)]
                atn = [[bp.tile([64, N], BF16, name=f"at{b}{h}", tag=f"at{b}{h}")
                        for h in range(2)] for b in range(B)]
                for b in range(B):
                    for h in range(2):
                        nc.vector.memset(vsb[b][h][:, :, 64:65], 1.0)

                # ---------------- Stage A: QKV + RMSNorm + RoPE + transposes ----
                for b in range(B):
                    for mt in range(NMACRO):
                        hs_t = []
                        for cc in range(8):
                            tf_ = sp.tile([128, 256], F32, name=f"hsf{cc}", tag=f"hsf{cc}")
                            nc.sync.dma_start(
                                tf_[:], hsT_d.ap()[cc * 128:(cc + 1) * 128,
                                                   b * N + mt * 256: b * N + (mt + 1) * 256])
                            t = sp.tile([128, 256], F32R, name=f"hs{cc}", tag=f"hs{cc}")
                            nc.vector.tensor_copy(t[:], tf_[:])
                            hs_t.append(t)
                        trig = {}
                        for nm, dt_ in (("cosq", cosq_d), ("sinq", sinq_d),
                                        ("cosk", cosk_d), ("sink", sink_d)):
                            tt_ = sp.tile([128, 2, D], F32, name=nm, tag=nm)
                            nc.sync.dma_start(
                                tt_[:], dt_.ap()[mt * 256:(mt + 1) * 256, :]
                                .rearrange("(s p) d -> p s d", p=128))
                            trig[nm] = tt_

                        for sub in range(2):
                            tt = mt * 2 + sub  # token tile index within batch
                            ps_qkv = ps1.tile([128, 384], F32, name="ps_qkv", tag="ps1")
                            for cc in range(8):
                                nc.tensor.matmul(
                                    ps_qkv[:],
                                    lhsT=hs_t[cc][:, sub * 128:(sub + 1) * 128],
                                    rhs=wqkv_sb[:, cc, :],
                                    start=(cc == 0), stop=(cc == 7))

                            for (base, cname, sname, dstname) in (
                                    (0, "cosq", "sinq", "q"), (128, "cosk", "sink", "k")):
                                d_tm = wp.tile([128, 128], F32, name=f"{dstname}_tm",
                                               tag=f"{dstname}_tm")
                                for hh in range(2):
                                    sl = ps_qkv[:, base + 64 * hh: base + 64 * (hh + 1)]
                                    sq = wp.tile([128, 64], F32, name="sq", tag="sq")
                                    ssq = wp.tile([128, 1], F32, name="ssq", tag="ssq")
                                    nc.scalar.activation(sq[:], sl, ACTF.Square,
                                                         accum_out=ssq[:])
                                    rstd = wp.tile([128, 1], F32, name="rstd", tag="rstd")
                                    nc.scalar.activation(rstd[:], ssq[:], ACTF.Sqrt,
                                                         bias=eps_sb[:], scale=1.0 / D)
                                    rinv = wp.tile([128, 1], F32, name="rinv", tag="rinv")
                                    nc.vector.reciprocal(rinv[:], rstd[:])
                                    tcos = wp.tile([128, 64], F32, name="tcos", tag="tcos")
                                    nc.vector.scalar_tensor_tensor(
                                        out=tcos[:], in0=sl, scalar=rinv[:],
                                        in1=trig[cname][:, sub, :],
                                        op0=ALU.mult, op1=ALU.mult)
                                    trot = wp.tile([128, 64], F32, name="trot", tag="trot")
                                    nc.vector.scalar_tensor_tensor(
                                        out=trot[:, 0:32], in0=sl[:, 32:64], scalar=rinv[:],
                                        in1=trig[sname][:, sub, 0:32],
                                        op0=ALU.mult, op1=ALU.mult)
                                    nc.vector.scalar_tensor_tensor(
                                        out=trot[:, 32:64], in0=sl[:, 0:32], scalar=rinv[:],
                                        in1=trig[sname][:, sub, 32:64],
                                        op0=ALU.mult, op1=ALU.mult)
                                    nc.vector.tensor_add(
                                        d_tm[:, 64 * hh:64 * (hh + 1)], tcos[:], trot[:])
                                # transpose [128 tok, 128 ch] -> [128 ch, 128 tok]
                                ps_t = ps1.tile([128, 128], F32, name="ps_t", tag="ps1")
                                nc.tensor.transpose(ps_t[:], d_tm[:], ident[:])
                                dst = qT[b] if dstname == "q" else kT[b]
                                nc.vector.tensor_copy(dst[:, tt * 128:(tt + 1) * 128], ps_t[:])
                            for hh in range(2):
                                nc.vector.tensor_copy(
                                    vsb[b][hh][:, tt, 0:64],
                                    ps_qkv[:, 256 + 64 * hh: 256 + 64 * (hh + 1)])

                # ---------------- Stage B: attention --------------------------
                for b in range(B):
                    for qt in range(NQT):
                        at_ps = [psat.tile([65, QTILE], F32, name=f"at_ps{h}", tag="psat")
                                 for h in range(2)]
                        for kc in range(KC):
                            for hh in range(2):
                                ps_s = pssc.tile([128, QTILE], F32, name="ps_s", tag="pssc")
                                nc.tensor.matmul(
                                    ps_s[:],
                                    lhsT=kT[b][64 * hh:64 * (hh + 1),
                                               kc * 128:(kc + 1) * 128],
                                    rhs=qT[b][64 * hh:64 * (hh + 1),
                                              qt * QTILE:(qt + 1) * QTILE],
                                    start=True, stop=True)
                                pr = pp.tile([128, QTILE], BF16, name="pr", tag="pr")
                                nc.scalar.activation(pr[:], ps_s[:], ACTF.Exp,
                                                     bias=neg3_sb[:], scale=0.125)
                                nc.tensor.matmul(
                                    at_ps[hh][:],
                                    lhsT=vsb[b][hh][:, kc, :],
                                    rhs=pr[:],
                                    start=(kc == 0), stop=(kc == KC - 1))
                        for hh in range(2):
                            # sums live in psum row 64; move to sbuf row 64, recip,
                            # PE-broadcast to 64 rows, then normalize.
                            smrow = wp.tile([65, QTILE], F32R, name="smrow", tag="smrow", bufs=2)
                            nc.vector.tensor_copy(smrow[64:65, :], at_ps[hh][64:65, :])
                            with nc.allow_low_precision(reason="softmax denom recip in f32r"):
                                nc.vector.reciprocal(smrow[64:65, :], smrow[64:65, :])
                            ps_bc = psbc.tile([64, QTILE], F32, name="ps_bc", tag="psbc")
                            nc.tensor.matmul(
                                ps_bc[:],
                                lhsT=ones_sb[64:65, :],
                                rhs=smrow[64:65, :],
                                start=True, stop=True)
                            rbc = wp.tile([64, QTILE], F32, name="rbc", tag="rbc", bufs=2)
                            nc.vector.tensor_copy(rbc[:], ps_bc[:])
                            nc.vector.tensor_mul(
                                atn[b][hh][:, qt * QTILE:(qt + 1) * QTILE],
                                at_ps[hh][0:64, :], rbc[:])

                # a2a input staging
                for d in range(NCORES):
                    bb, sh = d // 4, d % 4
                    nc.sync.dma_start(a2a_in[d, 0:64, :],
                                      atn[bb][0][:, sh * 1024:(sh + 1) * 1024])
                    nc.sync.dma_start(a2a_in[d, 64:128, :],
                                      atn[bb][1][:, sh * 1024:(sh + 1) * 1024])

            nc.gpsimd.collective_compute(
                "AllToAll", ALU.bypass,
                ins=[a2a_in[:].opt()], outs=[a2a_out[:].opt()],
                replica_groups=[list(range(NCORES))])

            # ---------------- Stage C: output projection ----------------------
            with tc.tile_pool(name="cstage", bufs=1) as cp, \
                 tc.tile_pool(name="cwork", bufs=2) as cw, \
                 tc.tile_pool(name="psC", bufs=2, space="PSUM") as psC:
                atf = cp.tile([128, 8, SHARD], BF16)
                nc.sync.dma_start(atf[:], a2a_out[:].transpose([1, 0, 2]))
                for ttk in range(SHARD // 128):
                    ostage = cw.tile([128, C], F32, name="ostage", tag="ostage")
                    for half in range(2):
                        ps_o = psC.tile([128, 512], F32, name="ps_o", tag="psC")
                        for cc in range(8):
                            nc.tensor.matmul(
                                ps_o[:],
                                lhsT=atf[:, cc, ttk * 128:(ttk + 1) * 128],
                                rhs=wout_sb[:, cc, half * 512:(half + 1) * 512],
                                start=(cc == 0), stop=(cc == 7))
                        nc.vector.tensor_copy(ostage[:, half * 512:(half + 1) * 512], ps_o[:])
                    nc.sync.dma_start(out_d.ap()[ttk * 128:(ttk + 1) * 128, :], ostage[:])

    nc.compile()
    return nc


def _fold_sin(sin, g):
    out = np.empty_like(sin)
    out[:, :32] = -sin[:, :32] * g[32:]
    out[:, 32:] = sin[:, 32:] * g[:32]
    return out


def kernel(hidden_states, cos, sin, Wqkv, Wout, gq, gk):
    global _LAST_RESULT
    _install_profile_shim()

    hidden_states = np.asarray(hidden_states, dtype=np.float32)
    cos = np.asarray(cos, dtype=np.float32)
    sin = np.asarray(sin, dtype=np.float32)
    Wqkv = np.asarray(Wqkv, dtype=np.float32)
    Wout = np.asarray(Wout, dtype=np.float32)
    gq = np.asarray(gq, dtype=np.float32)
    gk = np.asarray(gk, dtype=np.float32)

    if "nc" not in _CACHE:
        _CACHE["nc"] = _build_graph()
    nc = _CACHE["nc"]

    hsT = np.ascontiguousarray(hidden_states.reshape(TOK, C).T)
    cosq = np.ascontiguousarray(cos * gq[None, :])
    sinq = _fold_sin(sin, gq)
    cosk = np.ascontiguousarray(cos * gk[None, :])
    sink = _fold_sin(sin, gk)

    in_maps = []
    for c in range(NCORES):
        wq = Wqkv[:, c * 128:(c + 1) * 128]
        wk = Wqkv[:, C + c * 128:C + (c + 1) * 128]
        wv = Wqkv[:, 2 * C + c * 128:2 * C + (c + 1) * 128]
        wqkv_loc = np.ascontiguousarray(np.concatenate([wq, wk, wv], axis=1))
        in_maps.append({
            "hsT": hsT, "wqkv": wqkv_loc, "cosq": cosq, "sinq": sinq,
            "cosk": cosk, "sink": sink, "wout": Wout,
        })

    trace = bool(os.environ.get("BASS_TRACE"))
    res = run_bass_kernel_spmd(nc, in_maps, core_ids=list(range(NCORES)), trace=trace)
    _LAST_RESULT = res

    out = np.concatenate([res.results[c]["out"] for c in range(NCORES)], axis=0)
    return out.reshape(B, N, C)


# revision 14
# speedup vs baseline: 1.2992x; 1.2992x over previous
"""Trainium2 8-core fused attention kernel (QKV proj + RMSNorm + RoPE + SDPA + out proj).

Sharding: tensor-parallel over heads. Each of the 8 cores computes 2 of the 16
heads end-to-end (QKV projection with its Wqkv column shard, per-head RMSNorm +
RoPE, full softmax attention), then an AllToAll redistributes the per-head
attention outputs so every core holds all 1024 attention channels for 1/8 of
the tokens and applies the full Wout to its token shard.

Self-contained: hardcodes all shapes from the problem spec.
"""
import os
import sys
import types

import numpy as np

sys.path.insert(0, "/opt/trn_rl_repo")

from concourse import bass, bacc, tile, mybir  # noqa: E402
from concourse.bass_utils import run_bass_kernel_spmd  # noqa: E402
from concourse.masks import make_identity  # noqa: E402

B, N, C, H, D = 2, 4096, 1024, 16, 64
NCORES = 8
TOK = B * N            # 8192 global tokens
NB = N // 128          # 32 token tiles per batch
NMACRO = N // 256      # 16 macro tiles (256 tok) per batch
QTILE = 512
NQT = N // QTILE       # 8 q tiles per batch
KC = N // 128          # 32 key chunks per batch
SHARD = TOK // NCORES  # 1024 tokens per core shard
EPS = 1e-6

F32 = mybir.dt.float32
F32R = mybir.dt.float32r
BF16 = mybir.dt.bfloat16
ALU = mybir.AluOpType
ACTF = mybir.ActivationFunctionType

_CACHE = {}
_LAST_RESULT = None


def _install_profile_shim():
    """trn_boot skips the NTFF hook when antenv.axon_hooks is missing; supply it."""
    try:
        import antenv
        if getattr(antenv, "axon_hooks", None) is not None:
            return
        from trn_agent_boot.trn_boot import _ntff_profile_via_ctypes
        hook = _ntff_profile_via_ctypes("/opt/axon/libaxon_pjrt.so")
        if hook is None:
            return
        mod = types.ModuleType("antenv.axon_hooks")
        state = {"hook": hook}
        mod.get_axon_ntff_profile_hook = lambda: state["hook"]
        mod.set_axon_ntff_profile_hook = lambda h: state.__setitem__("hook", h)
        sys.modules["antenv.axon_hooks"] = mod
        antenv.axon_hooks = mod
    except Exception:
        pass


def _build_graph():
    nc = bacc.Bacc("TRN2", target_bir_lowering=False, debug=False,
                   enable_asserts=True, num_devices=NCORES)

    hsT_d = nc.dram_tensor("hsT", [C, TOK], F32, kind="ExternalInput")
    wqkv_d = nc.dram_tensor("wqkv", [C, 384], F32, kind="ExternalInput")
    cosq_d = nc.dram_tensor("cosq", [N, D], F32, kind="ExternalInput")
    sinq_d = nc.dram_tensor("sinq", [N, D], F32, kind="ExternalInput")
    cosk_d = nc.dram_tensor("cosk", [N, D], F32, kind="ExternalInput")
    sink_d = nc.dram_tensor("sink", [N, D], F32, kind="ExternalInput")
    wout_d = nc.dram_tensor("wout", [C, C], F32, kind="ExternalInput")
    out_d = nc.dram_tensor("out", [SHARD, C], F32, kind="ExternalOutput")

    with tile.TileContext(nc) as tc:
        with tc.tile_pool(name="const", bufs=1) as constp, \
             tc.tile_pool(name="dram", bufs=1, space="DRAM") as dram:
            # resident weights
            wqkv_sb = constp.tile([128, 8, 384], F32R)
            wout_sb = constp.tile([128, 8, C], BF16)
            with tc.tile_pool(name="wtmp", bufs=2) as wtmp:
                for cc in range(8):
                    wqt = wtmp.tile([128, 384], F32, name="wqt", tag="wqt")
                    nc.sync.dma_start(
                        wqt[:], wqkv_d.ap()[cc * 128:(cc + 1) * 128, :])
                    nc.vector.tensor_copy(wqkv_sb[:, cc, :], wqt[:])
                    wtt = wtmp.tile([128, C], F32, name="wtt", tag="wtt")
                    nc.sync.dma_start(
                        wtt[:], wout_d.ap()[cc * 128:(cc + 1) * 128, :])
                    nc.vector.tensor_copy(wout_sb[:, cc, :], wtt[:])
            ident = constp.tile([128, 128], F32)
            make_identity(nc, ident[:])
            ones_f = constp.tile([65, 64], F32)
            nc.vector.memset(ones_f[:], 1.0)
            ones_sb = constp.tile([65, 64], F32R)
            nc.vector.tensor_copy(ones_sb[:], ones_f[:])
            eps_sb = constp.tile([128, 1], F32)
            nc.vector.memset(eps_sb[:], EPS)
            neg3_sb = constp.tile([128, 1], F32)
            nc.vector.memset(neg3_sb[:], -3.0)

            a2a_in = dram.tile([NCORES, 128, SHARD], BF16)
            a2a_out = dram.tile([NCORES, 128, SHARD], BF16)

            with tc.tile_pool(name="batch", bufs=1) as bp, \
                 tc.tile_pool(name="stream", bufs=2) as sp, \
                 tc.tile_pool(name="work", bufs=3) as wp, \
                 tc.tile_pool(name="probsp", bufs=4) as pp, \
                 tc.tile_pool(name="ps1", bufs=2, space="PSUM") as ps1, \
                 tc.tile_pool(name="pssc", bufs=2, space="PSUM") as pssc, \
                 tc.tile_pool(name="psat", bufs=3, space="PSUM") as psat, \
                 tc.tile_pool(name="psbc", bufs=1, space="PSUM") as psbc:

                qT = [bp.tile([128, N], F32R, name=f"qT{b}", tag=f"qT{b}") for b in range(B)]
                kT = [bp.tile([128, N], F32R, name=f"kT{b}", tag=f"kT{b}") for b in range(B)]
                vsb = [[bp.tile([128, NB, 65], BF16, name=f"v{b}{h}", tag=f"v{b}{h}")
                        for h in range(2)] for b in range(B)]
                atn = [[bp.tile([64, N], BF16, name=f"at{b}{h}", tag=f"at{b}{h}")
                        for h in range(2)] for b in range(B)]
                for b in range(B):
                    for h in range(2):
                        nc.vector.memset(vsb[b][h][:, :, 64:65], 1.0)

                # ---------------- Stage A: QKV + RMSNorm + RoPE + transposes ----
                for b in range(B):
                    for mt in range(NMACRO):
                        hs_t = []
                        for cc in range(8):
                            tf_ = sp.tile([128, 256], F32, name=f"hsf{cc}", tag=f"hsf{cc}")
                            nc.sync.dma_start(
                                tf_[:], hsT_d.ap()[cc * 128:(cc + 1) * 128,
                                                   b * N + mt * 256: b * N + (mt + 1) * 256])
                            t = sp.tile([128, 256], F32R, name=f"hs{cc}", tag=f"hs{cc}")
                            nc.vector.tensor_copy(t[:], tf_[:])
                            hs_t.append(t)
                        trig = {}
                        for nm, dt_ in (("cosq", cosq_d), ("sinq", sinq_d),
                                        ("cosk", cosk_d), ("sink", sink_d)):
                            tt_ = sp.tile([128, 2, D], F32, name=nm, tag=nm)
                            nc.sync.dma_start(
                                tt_[:], dt_.ap()[mt * 256:(mt + 1) * 256, :]
                                .rearrange("(s p) d -> p s d", p=128))
                            trig[nm] = tt_

                        for sub in range(2):
                            tt = mt * 2 + sub  # token tile index within batch
                            ps_qkv = ps1.tile([128, 384], F32, name="ps_qkv", tag="ps1")
                            for cc in range(8):
                                nc.tensor.matmul(
                                    ps_qkv[:],
                                    lhsT=hs_t[cc][:, sub * 128:(sub + 1) * 128],
                                    rhs=wqkv_sb[:, cc, :],
                                    start=(cc == 0), stop=(cc == 7))

                            # q/k block to SBUF; all norm math on DVE (keep ACT exp-only)
                            qk_sb = wp.tile([128, 256], F32, name="qk_sb", tag="qk_sb")
                            nc.vector.tensor_copy(qk_sb[:], ps_qkv[:, 0:256])
                            for hh in range(2):
                                nc.vector.tensor_copy(
                                    vsb[b][hh][:, tt, 0:64],
                                    ps_qkv[:, 256 + 64 * hh: 256 + 64 * (hh + 1)])
                            # sumsq for (q h0, q h1, k h0, k h1) -> [128, 4]
                            sq = wp.tile([128, 64], F32, name="sq", tag="sq")
                            ssq4 = wp.tile([128, 4], F32, name="ssq4", tag="ssq4")
                            for idx in range(4):
                                sl = qk_sb[:, idx * 64:(idx + 1) * 64]
                                nc.vector.tensor_mul(sq[:], sl, sl)
                                nc.vector.tensor_reduce(
                                    ssq4[:, idx:idx + 1], sq[:],
                                    axis=mybir.AxisListType.X, op=ALU.add)
                            # rinv = 1/sqrt(ssq/64 + eps) via bit-trick + 2 Newton steps
                            xm = wp.tile([128, 4], F32, name="xm", tag="xm")
                            nc.vector.tensor_scalar(out=xm[:], in0=ssq4[:],
                                                    scalar1=1.0 / D, scalar2=EPS,
                                                    op0=ALU.mult, op1=ALU.add)
                            yv = wp.tile([128, 4], F32, name="yv", tag="yv")
                            with nc.allow_low_precision(reason="rsqrt newton seed"):
                                nc.vector.tensor_scalar(
                                    out=yv[:].bitcast(mybir.dt.int32),
                                    in0=xm[:].bitcast(mybir.dt.int32),
                                    scalar1=1, scalar2=None, op0=ALU.arith_shift_right)
                                nc.vector.tensor_scalar(
                                    out=yv[:].bitcast(mybir.dt.int32),
                                    in0=yv[:].bitcast(mybir.dt.int32),
                                    scalar1=-1, scalar2=0x5F3759DF,
                                    op0=ALU.mult, op1=ALU.add)
                            tn = wp.tile([128, 4], F32, name="tn", tag="tn")
                            for _ in range(2):
                                nc.vector.tensor_mul(tn[:], yv[:], yv[:])
                                nc.vector.tensor_mul(tn[:], tn[:], xm[:])
                                nc.vector.tensor_scalar(out=tn[:], in0=tn[:],
                                                        scalar1=-0.5, scalar2=1.5,
                                                        op0=ALU.mult, op1=ALU.add)
                                nc.vector.tensor_mul(yv[:], yv[:], tn[:])
                            for (base, cname, sname, dstname) in (
                                    (0, "cosq", "sinq", "q"), (128, "cosk", "sink", "k")):
                                d_tm = wp.tile([128, 128], F32, name=f"{dstname}_tm",
                                               tag=f"{dstname}_tm")
                                for hh in range(2):
                                    idx = (0 if base == 0 else 2) + hh
                                    sl = qk_sb[:, idx * 64:(idx + 1) * 64]
                                    rinv = yv[:, idx:idx + 1]
                                    tcos = wp.tile([128, 64], F32, name="tcos", tag="tcos")
                                    nc.vector.scalar_tensor_tensor(
                                        out=tcos[:], in0=sl, scalar=rinv,
                                        in1=trig[cname][:, sub, :],
                                        op0=ALU.mult, op1=ALU.mult)
                                    trot = wp.tile([128, 64], F32, name="trot", tag="trot")
                                    nc.vector.scalar_tensor_tensor(
                                        out=trot[:, 0:32], in0=sl[:, 32:64], scalar=rinv,
                                        in1=trig[sname][:, sub, 0:32],
                                        op0=ALU.mult, op1=ALU.mult)
                                    nc.vector.scalar_tensor_tensor(
                                        out=trot[:, 32:64], in0=sl[:, 0:32], scalar=rinv,
                                        in1=trig[sname][:, sub, 32:64],
                                        op0=ALU.mult, op1=ALU.mult)
                                    nc.vector.tensor_add(
                                        d_tm[:, 64 * hh:64 * (hh + 1)], tcos[:], trot[:])
                                # transpose [128 tok, 128 ch] -> [128 ch, 128 tok]
                                ps_t = ps1.tile([128, 128], F32, name="ps_t", tag="ps1")
                                nc.tensor.transpose(ps_t[:], d_tm[:], ident[:])
                                dst = qT[b] if dstname == "q" else kT[b]
                                nc.vector.tensor_copy(dst[:, tt * 128:(tt + 1) * 128], ps_t[:])

                # ---------------- Stage B: attention --------------------------
                for b in range(B):
                    for qt in range(NQT):
                        at_ps = [psat.tile([65, QTILE], F32, name=f"at_ps{h}", tag="psat")
                                 for h in range(2)]
                        for kc in range(KC):
                            for hh in range(2):
                                ps_s = pssc.tile([128, QTILE], F32, name="ps_s", tag="pssc")
                                nc.tensor.matmul(
                                    ps_s[:],
                                    lhsT=kT[b][64 * hh:64 * (hh + 1),
                                               kc * 128:(kc + 1) * 128],
                                    rhs=qT[b][64 * hh:64 * (hh + 1),
                                              qt * QTILE:(qt + 1) * QTILE],
                                    start=True, stop=True)
                                pr = pp.tile([128, QTILE], BF16, name="pr", tag="pr")
                                nc.scalar.activation(pr[:], ps_s[:], ACTF.Exp,
                                                     bias=neg3_sb[:], scale=0.125)
                                nc.tensor.matmul(
                                    at_ps[hh][:],
                                    lhsT=vsb[b][hh][:, kc, :],
                                    rhs=pr[:],
                                    start=(kc == 0), stop=(kc == KC - 1))
                        for hh in range(2):
                            # sums live in psum row 64; move to sbuf row 64, recip,
                            # PE-broadcast to 64 rows, then normalize.
                            smrow = wp.tile([65, QTILE], F32R, name="smrow", tag="smrow", bufs=2)
                            nc.vector.tensor_copy(smrow[64:65, :], at_ps[hh][64:65, :])
                            with nc.allow_low_precision(reason="softmax denom recip in f32r"):
                                nc.vector.reciprocal(smrow[64:65, :], smrow[64:65, :])
                            ps_bc = psbc.tile([64, QTILE], F32, name="ps_bc", tag="psbc")
                            nc.tensor.matmul(
                                ps_bc[:],
                                lhsT=ones_sb[64:65, :],
                                rhs=smrow[64:65, :],
                                start=True, stop=True)
                            rbc = wp.tile([64, QTILE], F32, name="rbc", tag="rbc", bufs=2)
                            nc.vector.tensor_copy(rbc[:], ps_bc[:])
                            nc.vector.tensor_mul(
                                atn[b][hh][:, qt * QTILE:(qt + 1) * QTILE],
                                at_ps[hh][0:64, :], rbc[:])

                # a2a input staging
                for d in range(NCORES):
                    bb, sh = d // 4, d % 4
                    nc.sync.dma_start(a2a_in[d, 0:64, :],
                                      atn[bb][0][:, sh * 1024:(sh + 1) * 1024])
                    nc.sync.dma_start(a2a_in[d, 64:128, :],
                                      atn[bb][1][:, sh * 1024:(sh + 1) * 1024])

            nc.gpsimd.collective_compute(
                "AllToAll", ALU.bypass,
                ins=[a2a_in[:].opt()], outs=[a2a_out[:].opt()],
                replica_groups=[list(range(NCORES))])

            # ---------------- Stage C: output projection ----------------------
            with tc.tile_pool(name="cstage", bufs=1) as cp, \
                 tc.tile_pool(name="cwork", bufs=2) as cw, \
                 tc.tile_pool(name="psC", bufs=2, space="PSUM") as psC:
                atf = cp.tile([128, 8, SHARD], BF16)
                nc.sync.dma_start(atf[:], a2a_out[:].transpose([1, 0, 2]))
                for ttk in range(SHARD // 128):
                    ostage = cw.tile([128, C], F32, name="ostage", tag="ostage")
                    for half in range(2):
                        ps_o = psC.tile([128, 512], F32, name="ps_o", tag="psC")
                        for cc in range(8):
                            nc.tensor.matmul(
                                ps_o[:],
                                lhsT=atf[:, cc, ttk * 128:(ttk + 1) * 128],
                                rhs=wout_sb[:, cc, half * 512:(half + 1) * 512],
                                start=(cc == 0), stop=(cc == 7))
                        nc.vector.tensor_copy(ostage[:, half * 512:(half + 1) * 512], ps_o[:])
                    nc.sync.dma_start(out_d.ap()[ttk * 128:(ttk + 1) * 128, :], ostage[:])

    nc.compile()
    return nc


def _fold_sin(sin, g):
    out = np.empty_like(sin)
    out[:, :32] = -sin[:, :32] * g[32:]
    out[:, 32:] = sin[:, 32:] * g[:32]
    return out


def kernel(hidden_states, cos, sin, Wqkv, Wout, gq, gk):
    global _LAST_RESULT
    _install_profile_shim()

    hidden_states = np.asarray(hidden_states, dtype=np.float32)
    cos = np.asarray(cos, dtype=np.float32)
    sin = np.asarray(sin, dtype=np.float32)
    Wqkv = np.asarray(Wqkv, dtype=np.float32)
    Wout = np.asarray(Wout, dtype=np.float32)
    gq = np.asarray(gq, dtype=np.float32)
    gk = np.asarray(gk, dtype=np.float32)

    if "nc" not in _CACHE:
        _CACHE["nc"] = _build_graph()
    nc = _CACHE["nc"]

    hsT = np.ascontiguousarray(hidden_states.reshape(TOK, C).T)
    cosq = np.ascontiguousarray(cos * gq[None, :])
    sinq = _fold_sin(sin, gq)
    cosk = np.ascontiguousarray(cos * gk[None, :])
    sink = _fold_sin(sin, gk)

    in_maps = []
    for c in range(NCORES):
        wq = Wqkv[:, c * 128:(c + 1) * 128]
        wk = Wqkv[:, C + c * 128:C + (c + 1) * 128]
        wv = Wqkv[:, 2 * C + c * 128:2 * C + (c + 1) * 128]
        wqkv_loc = np.ascontiguousarray(np.concatenate([wq, wk, wv], axis=1))
        in_maps.append({
            "hsT": hsT, "wqkv": wqkv_loc, "cosq": cosq, "sinq": sinq,
            "cosk": cosk, "sink": sink, "wout": Wout,
        })

    trace = bool(os.environ.get("BASS_TRACE"))
    res = run_bass_kernel_spmd(nc, in_maps, core_ids=list(range(NCORES)), trace=trace)
    _LAST_RESULT = res

    out = np.concatenate([res.results[c]["out"] for c in range(NCORES)], axis=0)
    return out.reshape(B, N, C)


# revision 15
# speedup vs baseline: 1.5337x; 1.1805x over previous
"""Trainium2 8-core fused attention kernel (QKV proj + RMSNorm + RoPE + SDPA + out proj).

Sharding: tensor-parallel over heads. Each of the 8 cores computes 2 of the 16
heads end-to-end (QKV projection with its Wqkv column shard, per-head RMSNorm +
RoPE, full softmax attention), then an AllToAll redistributes the per-head
attention outputs so every core holds all 1024 attention channels for 1/8 of
the tokens and applies the full Wout to its token shard.

Self-contained: hardcodes all shapes from the problem spec.
"""
import os
import sys
import types

import numpy as np

sys.path.insert(0, "/opt/trn_rl_repo")

from concourse import bass, bacc, tile, mybir  # noqa: E402
from concourse.bass_utils import run_bass_kernel_spmd  # noqa: E402
from concourse.masks import make_identity  # noqa: E402

B, N, C, H, D = 2, 4096, 1024, 16, 64
NCORES = 8
TOK = B * N            # 8192 global tokens
NB = N // 128          # 32 token tiles per batch
NMACRO = N // 256      # 16 macro tiles (256 tok) per batch
QTILE = 512
NQT = N // QTILE       # 8 q tiles per batch
KC = N // 128          # 32 key chunks per batch
SHARD = TOK // NCORES  # 1024 tokens per core shard
EPS = 1e-6

F32 = mybir.dt.float32
F32R = mybir.dt.float32r
BF16 = mybir.dt.bfloat16
ALU = mybir.AluOpType
ACTF = mybir.ActivationFunctionType

_CACHE = {}
_LAST_RESULT = None


def _install_profile_shim():
    """trn_boot skips the NTFF hook when antenv.axon_hooks is missing; supply it."""
    try:
        import antenv
        if getattr(antenv, "axon_hooks", None) is not None:
            return
        from trn_agent_boot.trn_boot import _ntff_profile_via_ctypes
        hook = _ntff_profile_via_ctypes("/opt/axon/libaxon_pjrt.so")
        if hook is None:
            return
        mod = types.ModuleType("antenv.axon_hooks")
        state = {"hook": hook}
        mod.get_axon_ntff_profile_hook = lambda: state["hook"]
        mod.set_axon_ntff_profile_hook = lambda h: state.__setitem__("hook", h)
        sys.modules["antenv.axon_hooks"] = mod
        antenv.axon_hooks = mod
    except Exception:
        pass


def _build_graph():
    nc = bacc.Bacc("TRN2", target_bir_lowering=False, debug=False,
                   enable_asserts=True, num_devices=NCORES)

    hsT_d = nc.dram_tensor("hsT", [C, TOK], F32, kind="ExternalInput")
    wqkv_d = nc.dram_tensor("wqkv", [C, 384], F32, kind="ExternalInput")
    cosq_d = nc.dram_tensor("cosq", [N, D], F32, kind="ExternalInput")
    sinq_d = nc.dram_tensor("sinq", [N, D], F32, kind="ExternalInput")
    cosk_d = nc.dram_tensor("cosk", [N, D], F32, kind="ExternalInput")
    sink_d = nc.dram_tensor("sink", [N, D], F32, kind="ExternalInput")
    wout_d = nc.dram_tensor("wout", [C, C], F32, kind="ExternalInput")
    out_d = nc.dram_tensor("out", [SHARD, C], F32, kind="ExternalOutput")

    with tile.TileContext(nc) as tc:
        with tc.tile_pool(name="const", bufs=1) as constp, \
             tc.tile_pool(name="dram", bufs=1, space="DRAM") as dram:
            # resident weights
            wqkv_sb = constp.tile([128, 8, 384], F32R)
            wout_sb = constp.tile([128, 8, C], BF16)
            with tc.tile_pool(name="wtmp", bufs=2) as wtmp:
                for cc in range(8):
                    wqt = wtmp.tile([128, 384], F32, name="wqt", tag="wqt")
                    nc.sync.dma_start(
                        wqt[:], wqkv_d.ap()[cc * 128:(cc + 1) * 128, :])
                    nc.vector.tensor_copy(wqkv_sb[:, cc, :], wqt[:])
                    wtt = wtmp.tile([128, C], F32, name="wtt", tag="wtt")
                    nc.sync.dma_start(
                        wtt[:], wout_d.ap()[cc * 128:(cc + 1) * 128, :])
                    nc.vector.tensor_copy(wout_sb[:, cc, :], wtt[:])
            ident = constp.tile([128, 128], F32)
            make_identity(nc, ident[:])
            ones_f = constp.tile([65, 64], F32)
            nc.vector.memset(ones_f[:], 1.0)
            ones_sb = constp.tile([65, 64], F32R)
            nc.vector.tensor_copy(ones_sb[:], ones_f[:])
            eps_sb = constp.tile([128, 1], F32)
            nc.vector.memset(eps_sb[:], EPS)
            neg3_sb = constp.tile([128, 1], F32)
            nc.vector.memset(neg3_sb[:], -3.0)

            a2a_in = dram.tile([NCORES, 128, SHARD], BF16)
            a2a_out = dram.tile([NCORES, 128, SHARD], BF16)

            with tc.tile_pool(name="batch", bufs=1) as bp, \
                 tc.tile_pool(name="stream", bufs=2) as sp, \
                 tc.tile_pool(name="work", bufs=3) as wp, \
                 tc.tile_pool(name="probsp", bufs=4) as pp, \
                 tc.tile_pool(name="ps1", bufs=2, space="PSUM") as ps1, \
                 tc.tile_pool(name="pssc", bufs=2, space="PSUM") as pssc, \
                 tc.tile_pool(name="psat", bufs=1, space="PSUM") as psat:

                qT = [bp.tile([128, N], F32R, name=f"qT{b}", tag=f"qT{b}") for b in range(B)]
                kT = [bp.tile([128, N], F32R, name=f"kT{b}", tag=f"kT{b}") for b in range(B)]
                vsb = [[bp.tile([128, NB, 65], BF16, name=f"v{b}{h}", tag=f"v{b}{h}")
                        for h in range(2)] for b in range(B)]
                atn = [[bp.tile([64, N], BF16, name=f"at{b}{h}", tag=f"at{b}{h}")
                        for h in range(2)] for b in range(B)]
                for b in range(B):
                    for h in range(2):
                        nc.vector.memset(vsb[b][h][:, :, 64:65], 1.0)

                # ---------------- Stage A: QKV + RMSNorm + RoPE + transposes ----
                for b in range(B):
                    for mt in range(NMACRO):
                        hs_t = []
                        for cc in range(8):
                            tf_ = sp.tile([128, 256], F32, name=f"hsf{cc}", tag=f"hsf{cc}")
                            nc.sync.dma_start(
                                tf_[:], hsT_d.ap()[cc * 128:(cc + 1) * 128,
                                                   b * N + mt * 256: b * N + (mt + 1) * 256])
                            t = sp.tile([128, 256], F32R, name=f"hs{cc}", tag=f"hs{cc}")
                            nc.vector.tensor_copy(t[:], tf_[:])
                            hs_t.append(t)
                        trig = {}
                        for nm, dt_ in (("cosq", cosq_d), ("sinq", sinq_d),
                                        ("cosk", cosk_d), ("sink", sink_d)):
                            tt_ = sp.tile([128, 2, D], F32, name=nm, tag=nm)
                            nc.sync.dma_start(
                                tt_[:], dt_.ap()[mt * 256:(mt + 1) * 256, :]
                                .rearrange("(s p) d -> p s d", p=128))
                            trig[nm] = tt_

                        for sub in range(2):
                            tt = mt * 2 + sub  # token tile index within batch
                            ps_qkv = ps1.tile([128, 384], F32, name="ps_qkv", tag="ps1")
                            for cc in range(8):
                                nc.tensor.matmul(
                                    ps_qkv[:],
                                    lhsT=hs_t[cc][:, sub * 128:(sub + 1) * 128],
                                    rhs=wqkv_sb[:, cc, :],
                                    start=(cc == 0), stop=(cc == 7))

                            # q/k block to SBUF; all norm math on DVE (keep ACT exp-only)
                            qk_sb = wp.tile([128, 256], F32, name="qk_sb", tag="qk_sb")
                            nc.vector.tensor_copy(qk_sb[:], ps_qkv[:, 0:256])
                            for hh in range(2):
                                nc.vector.tensor_copy(
                                    vsb[b][hh][:, tt, 0:64],
                                    ps_qkv[:, 256 + 64 * hh: 256 + 64 * (hh + 1)])
                            # sumsq for (q h0, q h1, k h0, k h1) -> [128, 4]
                            sq = wp.tile([128, 64], F32, name="sq", tag="sq")
                            ssq4 = wp.tile([128, 4], F32, name="ssq4", tag="ssq4")
                            for idx in range(4):
                                sl = qk_sb[:, idx * 64:(idx + 1) * 64]
                                nc.vector.tensor_mul(sq[:], sl, sl)
                                nc.vector.tensor_reduce(
                                    ssq4[:, idx:idx + 1], sq[:],
                                    axis=mybir.AxisListType.X, op=ALU.add)
                            # rinv = 1/sqrt(ssq/64 + eps) via bit-trick + 2 Newton steps
                            xm = wp.tile([128, 4], F32, name="xm", tag="xm")
                            nc.vector.tensor_scalar(out=xm[:], in0=ssq4[:],
                                                    scalar1=1.0 / D, scalar2=EPS,
                                                    op0=ALU.mult, op1=ALU.add)
                            yv = wp.tile([128, 4], F32, name="yv", tag="yv")
                            with nc.allow_low_precision(reason="rsqrt newton seed"):
                                nc.vector.tensor_scalar(
                                    out=yv[:].bitcast(mybir.dt.int32),
                                    in0=xm[:].bitcast(mybir.dt.int32),
                                    scalar1=1, scalar2=None, op0=ALU.arith_shift_right)
                                nc.vector.tensor_scalar(
                                    out=yv[:].bitcast(mybir.dt.int32),
                                    in0=yv[:].bitcast(mybir.dt.int32),
                                    scalar1=-1, scalar2=0x5F3759DF,
                                    op0=ALU.mult, op1=ALU.add)
                            tn = wp.tile([128, 4], F32, name="tn", tag="tn")
                            for _ in range(2):
                                nc.vector.tensor_mul(tn[:], yv[:], yv[:])
                                nc.vector.tensor_mul(tn[:], tn[:], xm[:])
                                nc.vector.tensor_scalar(out=tn[:], in0=tn[:],
                                                        scalar1=-0.5, scalar2=1.5,
                                                        op0=ALU.mult, op1=ALU.add)
                                nc.vector.tensor_mul(yv[:], yv[:], tn[:])
                            for (base, cname, sname, dstname) in (
                                    (0, "cosq", "sinq", "q"), (128, "cosk", "sink", "k")):
                                d_tm = wp.tile([128, 128], F32, name=f"{dstname}_tm",
                                               tag=f"{dstname}_tm")
                                for hh in range(2):
                                    idx = (0 if base == 0 else 2) + hh
                                    sl = qk_sb[:, idx * 64:(idx + 1) * 64]
                                    rinv = yv[:, idx:idx + 1]
                                    tcos = wp.tile([128, 64], F32, name="tcos", tag="tcos")
                                    nc.vector.scalar_tensor_tensor(
                                        out=tcos[:], in0=sl, scalar=rinv,
                                        in1=trig[cname][:, sub, :],
                                        op0=ALU.mult, op1=ALU.mult)
                                    trot = wp.tile([128, 64], F32, name="trot", tag="trot")
                                    nc.vector.scalar_tensor_tensor(
                                        out=trot[:, 0:32], in0=sl[:, 32:64], scalar=rinv,
                                        in1=trig[sname][:, sub, 0:32],
                                        op0=ALU.mult, op1=ALU.mult)
                                    nc.vector.scalar_tensor_tensor(
                                        out=trot[:, 32:64], in0=sl[:, 0:32], scalar=rinv,
                                        in1=trig[sname][:, sub, 32:64],
                                        op0=ALU.mult, op1=ALU.mult)
                                    nc.vector.tensor_add(
                                        d_tm[:, 64 * hh:64 * (hh + 1)], tcos[:], trot[:])
                                # transpose [128 tok, 128 ch] -> [128 ch, 128 tok]
                                ps_t = ps1.tile([128, 128], F32, name="ps_t", tag="ps1")
                                nc.tensor.transpose(ps_t[:], d_tm[:], ident[:])
                                dst = qT[b] if dstname == "q" else kT[b]
                                nc.vector.tensor_copy(dst[:, tt * 128:(tt + 1) * 128], ps_t[:])

                # ---------------- Stage B: attention --------------------------
                # software pipeline: emit scores(kc)+exp(kc), then attn(kc-1) so
                # the PE never waits on the ACT exp of the current chunk.
                for b in range(B):
                    for qt in range(NQT):
                        at_ps = psat.tile([65, 2 * QTILE], F32, name="at_ps", tag="psat",
                                          bufs=1)
                        prev_pr = None
                        for kc in range(KC):
                            ps_s = pssc.tile([128, 2 * QTILE], F32, name="ps_s", tag="pssc")
                            for hh in range(2):
                                nc.tensor.matmul(
                                    ps_s[:, hh * QTILE:(hh + 1) * QTILE],
                                    lhsT=kT[b][64 * hh:64 * (hh + 1),
                                               kc * 128:(kc + 1) * 128],
                                    rhs=qT[b][64 * hh:64 * (hh + 1),
                                              qt * QTILE:(qt + 1) * QTILE],
                                    start=True, stop=True)
                            pr = pp.tile([128, 2 * QTILE], BF16, name="pr", tag="pr",
                                         bufs=3)
                            nc.scalar.activation(pr[:], ps_s[:], ACTF.Exp,
                                                 bias=neg3_sb[:], scale=0.125)
                            if prev_pr is not None:
                                pkc, ppr = prev_pr
                                for hh in range(2):
                                    nc.tensor.matmul(
                                        at_ps[:, hh * QTILE:(hh + 1) * QTILE],
                                        lhsT=vsb[b][hh][:, pkc, :],
                                        rhs=ppr[:, hh * QTILE:(hh + 1) * QTILE],
                                        start=(pkc == 0), stop=(pkc == KC - 1))
                            prev_pr = (kc, pr)
                        pkc, ppr = prev_pr
                        for hh in range(2):
                            nc.tensor.matmul(
                                at_ps[:, hh * QTILE:(hh + 1) * QTILE],
                                lhsT=vsb[b][hh][:, pkc, :],
                                rhs=ppr[:, hh * QTILE:(hh + 1) * QTILE],
                                start=(pkc == 0), stop=(pkc == KC - 1))
                        for hh in range(2):
                            # sums live in psum row 64; move to sbuf row 64, recip,
                            # PE-broadcast to 64 rows, then normalize.
                            aps = at_ps[:, hh * QTILE:(hh + 1) * QTILE]
                            smrow = wp.tile([65, QTILE], F32R, name="smrow", tag="smrow", bufs=2)
                            nc.vector.tensor_copy(smrow[64:65, :], aps[64:65, :])
                            with nc.allow_low_precision(reason="softmax denom recip in f32r"):
                                nc.vector.reciprocal(smrow[64:65, :], smrow[64:65, :])
                            ps_bc = pssc.tile([64, QTILE], F32, name="ps_bc", tag="pssc")
                            nc.tensor.matmul(
                                ps_bc[:],
                                lhsT=ones_sb[64:65, :],
                                rhs=smrow[64:65, :],
                                start=True, stop=True)
                            rbc = wp.tile([64, QTILE], F32, name="rbc", tag="rbc", bufs=2)
                            nc.vector.tensor_copy(rbc[:], ps_bc[:])
                            nc.vector.tensor_mul(
                                atn[b][hh][:, qt * QTILE:(qt + 1) * QTILE],
                                aps[0:64, :], rbc[:])

                # a2a input staging
                for d in range(NCORES):
                    bb, sh = d // 4, d % 4
                    nc.sync.dma_start(a2a_in[d, 0:64, :],
                                      atn[bb][0][:, sh * 1024:(sh + 1) * 1024])
                    nc.sync.dma_start(a2a_in[d, 64:128, :],
                                      atn[bb][1][:, sh * 1024:(sh + 1) * 1024])

            nc.gpsimd.collective_compute(
                "AllToAll", ALU.bypass,
                ins=[a2a_in[:].opt()], outs=[a2a_out[:].opt()],
                replica_groups=[list(range(NCORES))])

            # ---------------- Stage C: output projection ----------------------
            with tc.tile_pool(name="cstage", bufs=1) as cp, \
                 tc.tile_pool(name="cwork", bufs=2) as cw, \
                 tc.tile_pool(name="psC", bufs=2, space="PSUM") as psC:
                atf = cp.tile([128, 8, SHARD], BF16)
                nc.sync.dma_start(atf[:], a2a_out[:].transpose([1, 0, 2]))
                for ttk in range(SHARD // 128):
                    ostage = cw.tile([128, C], F32, name="ostage", tag="ostage")
                    for half in range(2):
                        ps_o = psC.tile([128, 512], F32, name="ps_o", tag="psC")
                        for cc in range(8):
                            nc.tensor.matmul(
                                ps_o[:],
                                lhsT=atf[:, cc, ttk * 128:(ttk + 1) * 128],
                                rhs=wout_sb[:, cc, half * 512:(half + 1) * 512],
                                start=(cc == 0), stop=(cc == 7))
                        nc.vector.tensor_copy(ostage[:, half * 512:(half + 1) * 512], ps_o[:])
                    nc.sync.dma_start(out_d.ap()[ttk * 128:(ttk + 1) * 128, :], ostage[:])

    nc.compile()
    return nc


def _fold_sin(sin, g):
    out = np.empty_like(sin)
    out[:, :32] = -sin[:, :32] * g[32:]
    out[:, 32:] = sin[:, 32:] * g[:32]
    return out


def kernel(hidden_states, cos, sin, Wqkv, Wout, gq, gk):
    global _LAST_RESULT
    _install_profile_shim()

    hidden_states = np.asarray(hidden_states, dtype=np.float32)
    cos = np.asarray(cos, dtype=np.float32)
    sin = np.asarray(sin, dtype=np.float32)
    Wqkv = np.asarray(Wqkv, dtype=np.float32)
    Wout = np.asarray(Wout, dtype=np.float32)
    gq = np.asarray(gq, dtype=np.float32)
    gk = np.asarray(gk, dtype=np.float32)

    if "nc" not in _CACHE:
        _CACHE["nc"] = _build_graph()
    nc = _CACHE["nc"]

    hsT = np.ascontiguousarray(hidden_states.reshape(TOK, C).T)
    cosq = np.ascontiguousarray(cos * gq[None, :])
    sinq = _fold_sin(sin, gq)
    cosk = np.ascontiguousarray(cos * gk[None, :])
    sink = _fold_sin(sin, gk)

    in_maps = []
    for c in range(NCORES):
        wq = Wqkv[:, c * 128:(c + 1) * 128]
        wk = Wqkv[:, C + c * 128:C + (c + 1) * 128]
        wv = Wqkv[:, 2 * C + c * 128:2 * C + (c + 1) * 128]
        wqkv_loc = np.ascontiguousarray(np.concatenate([wq, wk, wv], axis=1))
        in_maps.append({
            "hsT": hsT, "wqkv": wqkv_loc, "cosq": cosq, "sinq": sinq,
            "cosk": cosk, "sink": sink, "wout": Wout,
        })

    trace = bool(os.environ.get("BASS_TRACE"))
    res = run_bass_kernel_spmd(nc, in_maps, core_ids=list(range(NCORES)), trace=trace)
    _LAST_RESULT = res

    out = np.concatenate([res.results[c]["out"] for c in range(NCORES)], axis=0)
    return out.reshape(B, N, C)


# revision 16
# speedup vs baseline: 1.7343x; 1.1308x over previous
"""Trainium2 8-core fused attention kernel (QKV proj + RMSNorm + RoPE + SDPA + out proj).

Sharding: tensor-parallel over heads. Each of the 8 cores computes 2 of the 16
heads end-to-end (QKV projection with its Wqkv column shard, per-head RMSNorm +
RoPE, full softmax attention), then an AllToAll redistributes the per-head
attention outputs so every core holds all 1024 attention channels for 1/8 of
the tokens and applies the full Wout to its token shard.

Self-contained: hardcodes all shapes from the problem spec.
"""
import os
import sys
import types

import numpy as np

sys.path.insert(0, "/opt/trn_rl_repo")

from concourse import bass, bacc, tile, mybir  # noqa: E402
from concourse.bass_utils import run_bass_kernel_spmd  # noqa: E402
from concourse.masks import make_identity  # noqa: E402

B, N, C, H, D = 2, 4096, 1024, 16, 64
NCORES = 8
TOK = B * N            # 8192 global tokens
NB = N // 128          # 32 token tiles per batch
NMACRO = N // 256      # 16 macro tiles (256 tok) per batch
QTILE = 512
NQT = N // QTILE       # 8 q tiles per batch
KC = N // 128          # 32 key chunks per batch
SHARD = TOK // NCORES  # 1024 tokens per core shard
EPS = 1e-6

F32 = mybir.dt.float32
F32R = mybir.dt.float32r
BF16 = mybir.dt.bfloat16
ALU = mybir.AluOpType
ACTF = mybir.ActivationFunctionType

_CACHE = {}
_LAST_RESULT = None


def _install_profile_shim():
    """trn_boot skips the NTFF hook when antenv.axon_hooks is missing; supply it."""
    try:
        import antenv
        if getattr(antenv, "axon_hooks", None) is not None:
            return
        from trn_agent_boot.trn_boot import _ntff_profile_via_ctypes
        hook = _ntff_profile_via_ctypes("/opt/axon/libaxon_pjrt.so")
        if hook is None:
            return
        mod = types.ModuleType("antenv.axon_hooks")
        state = {"hook": hook}
        mod.get_axon_ntff_profile_hook = lambda: state["hook"]
        mod.set_axon_ntff_profile_hook = lambda h: state.__setitem__("hook", h)
        sys.modules["antenv.axon_hooks"] = mod
        antenv.axon_hooks = mod
    except Exception:
        pass


def _build_graph():
    nc = bacc.Bacc("TRN2", target_bir_lowering=False, debug=False,
                   enable_asserts=True, num_devices=NCORES)

    hsT_d = nc.dram_tensor("hsT", [C, TOK], F32, kind="ExternalInput")
    wqkv_d = nc.dram_tensor("wqkv", [C, 384], F32, kind="ExternalInput")
    cosq_d = nc.dram_tensor("cosq", [N, D], F32, kind="ExternalInput")
    sinq_d = nc.dram_tensor("sinq", [N, D], F32, kind="ExternalInput")
    cosk_d = nc.dram_tensor("cosk", [N, D], F32, kind="ExternalInput")
    sink_d = nc.dram_tensor("sink", [N, D], F32, kind="ExternalInput")
    wout_d = nc.dram_tensor("wout", [C, C], F32, kind="ExternalInput")
    out_d = nc.dram_tensor("out", [SHARD, C], F32, kind="ExternalOutput")

    with tile.TileContext(nc) as tc:
        with tc.tile_pool(name="const", bufs=1) as constp, \
             tc.tile_pool(name="dram", bufs=1, space="DRAM") as dram:
            # resident weights
            wqkv_sb = constp.tile([128, 8, 384], F32R)
            wout_sb = constp.tile([128, 8, C], BF16)
            with tc.tile_pool(name="wtmp", bufs=2) as wtmp:
                for cc in range(8):
                    wqt = wtmp.tile([128, 384], F32, name="wqt", tag="wqt")
                    nc.sync.dma_start(
                        wqt[:], wqkv_d.ap()[cc * 128:(cc + 1) * 128, :])
                    nc.vector.tensor_copy(wqkv_sb[:, cc, :], wqt[:])
                    wtt = wtmp.tile([128, C], F32, name="wtt", tag="wtt")
                    nc.sync.dma_start(
                        wtt[:], wout_d.ap()[cc * 128:(cc + 1) * 128, :])
                    nc.vector.tensor_copy(wout_sb[:, cc, :], wtt[:])
            ident = constp.tile([128, 128], F32)
            make_identity(nc, ident[:])
            ones_f = constp.tile([65, 64], F32)
            nc.vector.memset(ones_f[:], 1.0)
            ones_sb = constp.tile([65, 64], F32R)
            nc.vector.tensor_copy(ones_sb[:], ones_f[:])
            eps_sb = constp.tile([128, 1], F32)
            nc.vector.memset(eps_sb[:], EPS)
            neg3_sb = constp.tile([128, 1], F32)
            nc.vector.memset(neg3_sb[:], -3.0)

            a2a_in = dram.tile([NCORES, 128, SHARD], BF16)
            a2a_out = dram.tile([NCORES, 128, SHARD], BF16)

            with tc.tile_pool(name="batch", bufs=1) as bp, \
                 tc.tile_pool(name="stream", bufs=2) as sp, \
                 tc.tile_pool(name="work", bufs=3) as wp, \
                 tc.tile_pool(name="probsp", bufs=4) as pp, \
                 tc.tile_pool(name="ps1", bufs=2, space="PSUM") as ps1, \
                 tc.tile_pool(name="pssc", bufs=2, space="PSUM") as pssc, \
                 tc.tile_pool(name="psat", bufs=1, space="PSUM") as psat:

                qT = [bp.tile([128, N], F32R, name=f"qT{b}", tag=f"qT{b}") for b in range(B)]
                kT = [bp.tile([128, N], F32R, name=f"kT{b}", tag=f"kT{b}") for b in range(B)]
                vsb = [bp.tile([128, NB, 2, 65], BF16, name=f"v{b}", tag=f"v{b}")
                       for b in range(B)]
                atn = [[bp.tile([64, N], BF16, name=f"at{b}{h}", tag=f"at{b}{h}")
                        for h in range(2)] for b in range(B)]
                for b in range(B):
                    nc.vector.memset(vsb[b][:, :, :, 64:65], 1.0)

                # ---------------- Stage A: QKV + RMSNorm + RoPE + transposes ----
                for b in range(B):
                    for mt in range(NMACRO):
                        hs_t = []
                        for cc in range(8):
                            tf_ = sp.tile([128, 256], F32, name=f"hsf{cc}", tag=f"hsf{cc}")
                            nc.sync.dma_start(
                                tf_[:], hsT_d.ap()[cc * 128:(cc + 1) * 128,
                                                   b * N + mt * 256: b * N + (mt + 1) * 256])
                            t = sp.tile([128, 256], F32R, name=f"hs{cc}", tag=f"hs{cc}")
                            nc.vector.tensor_copy(t[:], tf_[:])
                            hs_t.append(t)
                        trig = {}
                        for nm, dt_ in (("cosq", cosq_d), ("sinq", sinq_d),
                                        ("cosk", cosk_d), ("sink", sink_d)):
                            tt_ = sp.tile([128, 2, D], F32, name=nm, tag=nm)
                            nc.sync.dma_start(
                                tt_[:], dt_.ap()[mt * 256:(mt + 1) * 256, :]
                                .rearrange("(s p) d -> p s d", p=128))
                            trig[nm] = tt_

                        for sub in range(2):
                            tt = mt * 2 + sub  # token tile index within batch
                            ps_qkv = ps1.tile([128, 384], F32, name="ps_qkv", tag="ps1")
                            for cc in range(8):
                                nc.tensor.matmul(
                                    ps_qkv[:],
                                    lhsT=hs_t[cc][:, sub * 128:(sub + 1) * 128],
                                    rhs=wqkv_sb[:, cc, :],
                                    start=(cc == 0), stop=(cc == 7))

                            # q/k block to SBUF; all norm math on DVE (keep ACT exp-only)
                            qk_sb = wp.tile([128, 256], F32, name="qk_sb", tag="qk_sb")
                            nc.vector.tensor_copy(qk_sb[:], ps_qkv[:, 0:256])
                            nc.vector.tensor_copy(
                                vsb[b][:, tt, :, 0:64],
                                ps_qkv[:, 256:384].rearrange("p (h d) -> p h d", h=2))
                            # sumsq for (q h0, q h1, k h0, k h1) -> [128, 4]
                            sq = wp.tile([128, 256], F32, name="sq", tag="sq")
                            ssq4 = wp.tile([128, 4], F32, name="ssq4", tag="ssq4")
                            nc.vector.tensor_mul(sq[:], qk_sb[:], qk_sb[:])
                            nc.vector.tensor_reduce(
                                ssq4[:], sq[:].rearrange("p (a e) -> p a e", a=4),
                                axis=mybir.AxisListType.X, op=ALU.add)
                            # rinv = 1/sqrt(ssq/64 + eps) via bit-trick + 2 Newton steps
                            xm = wp.tile([128, 4], F32, name="xm", tag="xm")
                            nc.vector.tensor_scalar(out=xm[:], in0=ssq4[:],
                                                    scalar1=1.0 / D, scalar2=EPS,
                                                    op0=ALU.mult, op1=ALU.add)
                            yv = wp.tile([128, 4], F32, name="yv", tag="yv")
                            with nc.allow_low_precision(reason="rsqrt newton seed"):
                                nc.vector.tensor_scalar(
                                    out=yv[:].bitcast(mybir.dt.int32),
                                    in0=xm[:].bitcast(mybir.dt.int32),
                                    scalar1=1, scalar2=None, op0=ALU.arith_shift_right)
                                nc.vector.tensor_scalar(
                                    out=yv[:].bitcast(mybir.dt.int32),
                                    in0=yv[:].bitcast(mybir.dt.int32),
                                    scalar1=-1, scalar2=0x5F3759DF,
                                    op0=ALU.mult, op1=ALU.add)
                            tn = wp.tile([128, 4], F32, name="tn", tag="tn")
                            for _ in range(2):
                                nc.vector.tensor_mul(tn[:], yv[:], yv[:])
                                nc.vector.tensor_mul(tn[:], tn[:], xm[:])
                                nc.vector.tensor_scalar(out=tn[:], in0=tn[:],
                                                        scalar1=-0.5, scalar2=1.5,
                                                        op0=ALU.mult, op1=ALU.add)
                                nc.vector.tensor_mul(yv[:], yv[:], tn[:])
                            for (base, cname, sname, dstname) in (
                                    (0, "cosq", "sinq", "q"), (128, "cosk", "sink", "k")):
                                d_tm = wp.tile([128, 128], F32, name=f"{dstname}_tm",
                                               tag=f"{dstname}_tm")
                                for hh in range(2):
                                    idx = (0 if base == 0 else 2) + hh
                                    sl = qk_sb[:, idx * 64:(idx + 1) * 64]
                                    rinv = yv[:, idx:idx + 1]
                                    tcos = wp.tile([128, 64], F32, name="tcos", tag="tcos")
                                    nc.vector.scalar_tensor_tensor(
                                        out=tcos[:], in0=sl, scalar=rinv,
                                        in1=trig[cname][:, sub, :],
                                        op0=ALU.mult, op1=ALU.mult)
                                    trot = wp.tile([128, 64], F32, name="trot", tag="trot")
                                    nc.vector.scalar_tensor_tensor(
                                        out=trot[:, 0:32], in0=sl[:, 32:64], scalar=rinv,
                                        in1=trig[sname][:, sub, 0:32],
                                        op0=ALU.mult, op1=ALU.mult)
                                    nc.vector.scalar_tensor_tensor(
                                        out=trot[:, 32:64], in0=sl[:, 0:32], scalar=rinv,
                                        in1=trig[sname][:, sub, 32:64],
                                        op0=ALU.mult, op1=ALU.mult)
                                    nc.vector.tensor_add(
                                        d_tm[:, 64 * hh:64 * (hh + 1)], tcos[:], trot[:])
                                # transpose [128 tok, 128 ch] -> [128 ch, 128 tok]
                                ps_t = ps1.tile([128, 128], F32, name="ps_t", tag="ps1")
                                nc.tensor.transpose(ps_t[:], d_tm[:], ident[:])
                                dst = qT[b] if dstname == "q" else kT[b]
                                nc.vector.tensor_copy(dst[:, tt * 128:(tt + 1) * 128], ps_t[:])

                # ---------------- Stage B: attention --------------------------
                # software pipeline: emit scores(kc)+exp(kc), then attn(kc-1) so
                # the PE never waits on the ACT exp of the current chunk.
                for b in range(B):
                    for qt in range(NQT):
                        at_ps = psat.tile([65, 2 * QTILE], F32, name="at_ps", tag="psat",
                                          bufs=1)
                        prev_pr = None
                        for kc in range(KC):
                            ps_s = pssc.tile([128, 2 * QTILE], F32, name="ps_s", tag="pssc")
                            for hh in range(2):
                                nc.tensor.matmul(
                                    ps_s[:, hh * QTILE:(hh + 1) * QTILE],
                                    lhsT=kT[b][64 * hh:64 * (hh + 1),
                                               kc * 128:(kc + 1) * 128],
                                    rhs=qT[b][64 * hh:64 * (hh + 1),
                                              qt * QTILE:(qt + 1) * QTILE],
                                    start=True, stop=True)
                            pr = pp.tile([128, 2 * QTILE], BF16, name="pr", tag="pr",
                                         bufs=3)
                            nc.scalar.activation(pr[:], ps_s[:], ACTF.Exp,
                                                 bias=0.0, scale=0.125)
                            if prev_pr is not None:
                                pkc, ppr = prev_pr
                                for hh in range(2):
                                    nc.tensor.matmul(
                                        at_ps[:, hh * QTILE:(hh + 1) * QTILE],
                                        lhsT=vsb[b][:, pkc, hh, :],
                                        rhs=ppr[:, hh * QTILE:(hh + 1) * QTILE],
                                        start=(pkc == 0), stop=(pkc == KC - 1))
                            prev_pr = (kc, pr)
                        pkc, ppr = prev_pr
                        for hh in range(2):
                            nc.tensor.matmul(
                                at_ps[:, hh * QTILE:(hh + 1) * QTILE],
                                lhsT=vsb[b][:, pkc, hh, :],
                                rhs=ppr[:, hh * QTILE:(hh + 1) * QTILE],
                                start=(pkc == 0), stop=(pkc == KC - 1))
                        for hh in range(2):
                            # sums live in psum row 64; move to sbuf row 64, recip,
                            # PE-broadcast to 64 rows, then normalize.
                            aps = at_ps[:, hh * QTILE:(hh + 1) * QTILE]
                            smrow = wp.tile([65, QTILE], F32R, name="smrow", tag="smrow", bufs=2)
                            nc.vector.tensor_copy(smrow[64:65, :], aps[64:65, :])
                            ps_bc = pssc.tile([64, QTILE], F32, name="ps_bc", tag="pssc")
                            nc.tensor.matmul(
                                ps_bc[:],
                                lhsT=ones_sb[64:65, :],
                                rhs=smrow[64:65, :],
                                start=True, stop=True)
                            rbc = wp.tile([64, QTILE], F32, name="rbc", tag="rbc", bufs=2)
                            nc.vector.tensor_copy(rbc[:], ps_bc[:])
                            rbcr = wp.tile([64, QTILE], F32, name="rbcr", tag="rbcr", bufs=2)
                            nc.vector.reciprocal_approx_fast(out=rbcr[:], in_=rbc[:])
                            nc.vector.tensor_mul(
                                atn[b][hh][:, qt * QTILE:(qt + 1) * QTILE],
                                aps[0:64, :], rbcr[:])

                # a2a input staging
                for d in range(NCORES):
                    bb, sh = d // 4, d % 4
                    nc.sync.dma_start(a2a_in[d, 0:64, :],
                                      atn[bb][0][:, sh * 1024:(sh + 1) * 1024])
                    nc.sync.dma_start(a2a_in[d, 64:128, :],
                                      atn[bb][1][:, sh * 1024:(sh + 1) * 1024])

            nc.gpsimd.collective_compute(
                "AllToAll", ALU.bypass,
                ins=[a2a_in[:].opt()], outs=[a2a_out[:].opt()],
                replica_groups=[list(range(NCORES))])

            # ---------------- Stage C: output projection ----------------------
            with tc.tile_pool(name="cstage", bufs=1) as cp, \
                 tc.tile_pool(name="cwork", bufs=2) as cw, \
                 tc.tile_pool(name="psC", bufs=2, space="PSUM") as psC:
                atf = cp.tile([128, 8, SHARD], BF16)
                nc.sync.dma_start(atf[:], a2a_out[:].transpose([1, 0, 2]))
                for ttk in range(SHARD // 128):
                    ostage = cw.tile([128, C], F32, name="ostage", tag="ostage")
                    for half in range(2):
                        ps_o = psC.tile([128, 512], F32, name="ps_o", tag="psC")
                        for cc in range(8):
                            nc.tensor.matmul(
                                ps_o[:],
                                lhsT=atf[:, cc, ttk * 128:(ttk + 1) * 128],
                                rhs=wout_sb[:, cc, half * 512:(half + 1) * 512],
                                start=(cc == 0), stop=(cc == 7))
                        nc.vector.tensor_copy(ostage[:, half * 512:(half + 1) * 512], ps_o[:])
                    nc.sync.dma_start(out_d.ap()[ttk * 128:(ttk + 1) * 128, :], ostage[:])

    nc.compile()
    return nc


def _fold_sin(sin, g):
    out = np.empty_like(sin)
    out[:, :32] = -sin[:, :32] * g[32:]
    out[:, 32:] = sin[:, 32:] * g[:32]
    return out


def kernel(hidden_states, cos, sin, Wqkv, Wout, gq, gk):
    global _LAST_RESULT
    _install_profile_shim()

    hidden_states = np.asarray(hidden_states, dtype=np.float32)
    cos = np.asarray(cos, dtype=np.float32)
    sin = np.asarray(sin, dtype=np.float32)
    Wqkv = np.asarray(Wqkv, dtype=np.float32)
    Wout = np.asarray(Wout, dtype=np.float32)
    gq = np.asarray(gq, dtype=np.float32)
    gk = np.asarray(gk, dtype=np.float32)

    if "nc" not in _CACHE:
        _CACHE["nc"] = _build_graph()
    nc = _CACHE["nc"]

    hsT = np.ascontiguousarray(hidden_states.reshape(TOK, C).T)
    cosq = np.ascontiguousarray(cos * gq[None, :])
    sinq = _fold_sin(sin, gq)
    cosk = np.ascontiguousarray(cos * gk[None, :])
    sink = _fold_sin(sin, gk)

    in_maps = []
    for c in range(NCORES):
        wq = Wqkv[:, c * 128:(c + 1) * 128]
        wk = Wqkv[:, C + c * 128:C + (c + 1) * 128]
        wv = Wqkv[:, 2 * C + c * 128:2 * C + (c + 1) * 128]
        wqkv_loc = np.ascontiguousarray(np.concatenate([wq, wk, wv], axis=1))
        in_maps.append({
            "hsT": hsT, "wqkv": wqkv_loc, "cosq": cosq, "sinq": sinq,
            "cosk": cosk, "sink": sink, "wout": Wout,
        })

    trace = bool(os.environ.get("BASS_TRACE"))
    res = run_bass_kernel_spmd(nc, in_maps, core_ids=list(range(NCORES)), trace=trace)
    _LAST_RESULT = res

    out = np.concatenate([res.results[c]["out"] for c in range(NCORES)], axis=0)
    return out.reshape(B, N, C)


# revision 20
# speedup vs baseline: 1.8171x; 1.0478x over previous
"""Trainium2 8-core fused attention kernel (QKV proj + RMSNorm + RoPE + SDPA + out proj).

Sharding: tensor-parallel over heads. Each of the 8 cores computes 2 of the 16
heads end-to-end (QKV projection with its Wqkv column shard, per-head RMSNorm +
RoPE, full softmax attention), then an AllToAll redistributes the per-head
attention outputs so every core holds all 1024 attention channels for 1/8 of
the tokens and applies the full Wout to its token shard.

Self-contained: hardcodes all shapes from the problem spec.
"""
import os
import sys
import types

import numpy as np

sys.path.insert(0, "/opt/trn_rl_repo")

from concourse import bass, bacc, tile, mybir  # noqa: E402
from concourse.bass_utils import run_bass_kernel_spmd  # noqa: E402
from concourse.masks import make_identity  # noqa: E402

B, N, C, H, D = 2, 4096, 1024, 16, 64
NCORES = 8
TOK = B * N            # 8192 global tokens
NB = N // 128          # 32 token tiles per batch
NMACRO = N // 256      # 16 macro tiles (256 tok) per batch
QTILE = 512
NQT = N // QTILE       # 8 q tiles per batch
KC = N // 128          # 32 key chunks per batch
SHARD = TOK // NCORES  # 1024 tokens per core shard
EPS = 1e-6

F32 = mybir.dt.float32
F32R = mybir.dt.float32r
BF16 = mybir.dt.bfloat16
ALU = mybir.AluOpType
ACTF = mybir.ActivationFunctionType

_CACHE = {}
_LAST_RESULT = None


def _install_profile_shim():
    """trn_boot skips the NTFF hook when antenv.axon_hooks is missing; supply it."""
    try:
        import antenv
        if getattr(antenv, "axon_hooks", None) is not None:
            return
        from trn_agent_boot.trn_boot import _ntff_profile_via_ctypes
        hook = _ntff_profile_via_ctypes("/opt/axon/libaxon_pjrt.so")
        if hook is None:
            return
        mod = types.ModuleType("antenv.axon_hooks")
        state = {"hook": hook}
        mod.get_axon_ntff_profile_hook = lambda: state["hook"]
        mod.set_axon_ntff_profile_hook = lambda h: state.__setitem__("hook", h)
        sys.modules["antenv.axon_hooks"] = mod
        antenv.axon_hooks = mod
    except Exception:
        pass


def _build_graph():
    nc = bacc.Bacc("TRN2", target_bir_lowering=False, debug=False,
                   enable_asserts=True, num_devices=NCORES)

    hsT_d = nc.dram_tensor("hsT", [C, TOK], F32, kind="ExternalInput")
    wqkv_d = nc.dram_tensor("wqkv", [C, 384], F32, kind="ExternalInput")
    cosq_d = nc.dram_tensor("cosq", [N, D], F32, kind="ExternalInput")
    sinq_d = nc.dram_tensor("sinq", [N, D], F32, kind="ExternalInput")
    cosk_d = nc.dram_tensor("cosk", [N, D], F32, kind="ExternalInput")
    sink_d = nc.dram_tensor("sink", [N, D], F32, kind="ExternalInput")
    wout_d = nc.dram_tensor("wout", [C, C], F32, kind="ExternalInput")
    out_d = nc.dram_tensor("out", [SHARD, C], F32, kind="ExternalOutput")

    with tile.TileContext(nc) as tc:
        with tc.tile_pool(name="const", bufs=1) as constp, \
             tc.tile_pool(name="dram", bufs=1, space="DRAM") as dram:
            # resident weights
            wqkv_sb = constp.tile([128, 8, 384], F32R)
            wout_sb = constp.tile([128, 8, C], BF16)
            with tc.tile_pool(name="wtmp", bufs=2) as wtmp:
                for cc in range(8):
                    wqt = wtmp.tile([128, 384], F32, name="wqt", tag="wqt")
                    nc.sync.dma_start(
                        wqt[:], wqkv_d.ap()[cc * 128:(cc + 1) * 128, :])
                    nc.vector.tensor_copy(wqkv_sb[:, cc, :], wqt[:])
                    wtt = wtmp.tile([128, C], F32, name="wtt", tag="wtt")
                    nc.sync.dma_start(
                        wtt[:], wout_d.ap()[cc * 128:(cc + 1) * 128, :])
                    nc.vector.tensor_copy(wout_sb[:, cc, :], wtt[:])
            ident = constp.tile([128, 128], F32)
            make_identity(nc, ident[:])
            ones_f = constp.tile([65, 64], F32)
            nc.vector.memset(ones_f[:], 1.0)
            ones_sb = constp.tile([65, 64], F32R)
            nc.vector.tensor_copy(ones_sb[:], ones_f[:])
            eps_sb = constp.tile([128, 1], F32)
            nc.vector.memset(eps_sb[:], EPS)
            neg3_sb = constp.tile([128, 1], F32)
            nc.vector.memset(neg3_sb[:], -3.0)

            a2a_in = dram.tile([NCORES, 128, SHARD], BF16)
            a2a_out = dram.tile([NCORES, 128, SHARD], BF16)

            with tc.tile_pool(name="batch", bufs=1) as bp, \
                 tc.tile_pool(name="stream", bufs=2) as sp, \
                 tc.tile_pool(name="work", bufs=3) as wp, \
                 tc.tile_pool(name="probsp", bufs=4) as pp, \
                 tc.tile_pool(name="ps1", bufs=2, space="PSUM") as ps1, \
                 tc.tile_pool(name="pssc", bufs=2, space="PSUM") as pssc, \
                 tc.tile_pool(name="psat", bufs=1, space="PSUM") as psat:

                qT = [bp.tile([128, N], F32R, name=f"qT{b}", tag=f"qT{b}") for b in range(B)]
                kT = [bp.tile([128, N], F32R, name=f"kT{b}", tag=f"kT{b}") for b in range(B)]
                vsb = [bp.tile([128, NB, 2, 65], BF16, name=f"v{b}", tag=f"v{b}")
                       for b in range(B)]
                atn = [[bp.tile([64, N], BF16, name=f"at{b}{h}", tag=f"at{b}{h}")
                        for h in range(2)] for b in range(B)]
                for b in range(B):
                    nc.vector.memset(vsb[b][:, :, :, 64:65], 1.0)

                # ---------------- Stage A: QKV + RMSNorm + RoPE + transposes ----
                for b in range(B):
                    for mt in range(NMACRO):
                        hs_t = []
                        for cc in range(8):
                            tf_ = sp.tile([128, 256], F32, name=f"hsf{cc}", tag=f"hsf{cc}")
                            nc.sync.dma_start(
                                tf_[:], hsT_d.ap()[cc * 128:(cc + 1) * 128,
                                                   b * N + mt * 256: b * N + (mt + 1) * 256])
                            t = sp.tile([128, 256], F32R, name=f"hs{cc}", tag=f"hs{cc}")
                            nc.vector.tensor_copy(t[:], tf_[:])
                            hs_t.append(t)
                        trigC = sp.tile([128, 2, 256], F32, name="trigC", tag="trigC")
                        trigS = sp.tile([128, 2, 256], F32, name="trigS", tag="trigS")
                        for dst, dt_, off in ((trigC, cosq_d, 0), (trigC, cosk_d, 128),
                                              (trigS, sinq_d, 0), (trigS, sink_d, 128)):
                            for rep in range(2):
                                nc.sync.dma_start(
                                    dst[:, :, off + rep * D:off + (rep + 1) * D],
                                    dt_.ap()[mt * 256:(mt + 1) * 256, :]
                                    .rearrange("(s p) d -> p s d", p=128))

                        for sub in range(2):
                            tt = mt * 2 + sub  # token tile index within batch
                            ps_qkv = ps1.tile([128, 384], F32, name="ps_qkv", tag="ps1")
                            for cc in range(8):
                                nc.tensor.matmul(
                                    ps_qkv[:],
                                    lhsT=hs_t[cc][:, sub * 128:(sub + 1) * 128],
                                    rhs=wqkv_sb[:, cc, :],
                                    start=(cc == 0), stop=(cc == 7))

                            # q/k block to SBUF; all norm math on DVE (keep ACT exp-only)
                            qk_sb = wp.tile([128, 256], F32, name="qk_sb", tag="qk_sb", bufs=2)
                            nc.vector.tensor_copy(qk_sb[:], ps_qkv[:, 0:256])
                            nc.vector.tensor_copy(
                                vsb[b][:, tt, :, 0:64],
                                ps_qkv[:, 256:384].rearrange("p (h d) -> p h d", h=2))
                            # sumsq for (q h0, q h1, k h0, k h1) -> [128, 4]
                            sq = wp.tile([128, 256], F32, name="sq", tag="sq", bufs=2)
                            ssq4 = wp.tile([128, 4], F32, name="ssq4", tag="ssq4")
                            nc.vector.tensor_mul(sq[:], qk_sb[:], qk_sb[:])
                            nc.vector.tensor_reduce(
                                ssq4[:], sq[:].rearrange("p (a e) -> p a e", a=4),
                                axis=mybir.AxisListType.X, op=ALU.add)
                            # rinv = 1/sqrt(ssq/64 + eps): bit-trick seed + 1 Newton step
                            xm = wp.tile([128, 4], F32, name="xm", tag="xm")
                            nc.vector.tensor_scalar(out=xm[:], in0=ssq4[:],
                                                    scalar1=1.0 / D, scalar2=EPS,
                                                    op0=ALU.mult, op1=ALU.add)
                            yv = wp.tile([128, 4], F32, name="yv", tag="yv")
                            with nc.allow_low_precision(reason="rsqrt newton seed"):
                                nc.vector.tensor_scalar(
                                    out=yv[:].bitcast(mybir.dt.int32),
                                    in0=xm[:].bitcast(mybir.dt.int32),
                                    scalar1=1, scalar2=None, op0=ALU.arith_shift_right)
                                nc.vector.tensor_scalar(
                                    out=yv[:].bitcast(mybir.dt.int32),
                                    in0=yv[:].bitcast(mybir.dt.int32),
                                    scalar1=-1, scalar2=0x5F3759DF,
                                    op0=ALU.mult, op1=ALU.add)
                            tn = wp.tile([128, 4], F32, name="tn", tag="tn")
                            nc.vector.tensor_mul(tn[:], yv[:], yv[:])
                            nc.vector.tensor_mul(tn[:], tn[:], xm[:])
                            nc.vector.tensor_scalar(out=tn[:], in0=tn[:],
                                                    scalar1=-0.5, scalar2=1.5,
                                                    op0=ALU.mult, op1=ALU.add)
                            nc.vector.tensor_mul(yv[:], yv[:], tn[:])
                            # normalize all 4 groups, then wide rope over [128, 256]
                            qn2 = wp.tile([128, 256], F32, name="qn2", tag="qn2", bufs=2)
                            for g in range(4):
                                nc.vector.tensor_scalar(
                                    out=qn2[:, g * 64:(g + 1) * 64],
                                    in0=qk_sb[:, g * 64:(g + 1) * 64],
                                    scalar1=yv[:, g:g + 1], scalar2=None, op0=ALU.mult)
                            d_qk = wp.tile([128, 256], F32, name="d_qk", tag="d_qk", bufs=2)
                            nc.vector.tensor_mul(d_qk[:], qn2[:], trigC[:, sub, :])
                            trot = wp.tile([128, 256], F32, name="trot", tag="trot", bufs=2)
                            v4 = qn2[:].rearrange("p (a e) -> p a e", a=8)
                            s4 = trigS[:, sub, :].rearrange("p (a e) -> p a e", a=8)
                            t4 = trot[:].rearrange("p (a e) -> p a e", a=8)
                            nc.vector.tensor_mul(t4[:, 0:8:2, :], v4[:, 1:8:2, :],
                                                 s4[:, 0:8:2, :])
                            nc.vector.tensor_mul(t4[:, 1:8:2, :], v4[:, 0:8:2, :],
                                                 s4[:, 1:8:2, :])
                            nc.vector.tensor_add(d_qk[:], d_qk[:], trot[:])
                            for half, dstname in ((0, "q"), (1, "k")):
                                ps_t = ps1.tile([128, 128], F32, name="ps_t", tag="ps1")
                                nc.tensor.transpose(
                                    ps_t[:], d_qk[:, half * 128:(half + 1) * 128], ident[:])
                                dst = qT[b] if dstname == "q" else kT[b]
                                nc.vector.tensor_copy(dst[:, tt * 128:(tt + 1) * 128], ps_t[:])

                # ---------------- Stage B: attention --------------------------
                # software pipeline: emit scores(kc)+exp(kc), then attn(kc-1) so
                # the PE never waits on the ACT exp of the current chunk.
                for b in range(B):
                    for qt in range(NQT):
                        at_ps = psat.tile([65, 2 * QTILE], F32, name="at_ps", tag="psat",
                                          bufs=1)
                        prev_pr = None
                        for kc in range(KC):
                            ps_s = pssc.tile([128, 2 * QTILE], F32, name="ps_s", tag="pssc")
                            for hh in range(2):
                                nc.tensor.matmul(
                                    ps_s[:, hh * QTILE:(hh + 1) * QTILE],
                                    lhsT=kT[b][64 * hh:64 * (hh + 1),
                                               kc * 128:(kc + 1) * 128],
                                    rhs=qT[b][64 * hh:64 * (hh + 1),
                                              qt * QTILE:(qt + 1) * QTILE],
                                    start=True, stop=True)
                            pr = pp.tile([128, 2 * QTILE], BF16, name="pr", tag="pr",
                                         bufs=3)
                            nc.scalar.activation(pr[:], ps_s[:], ACTF.Exp,
                                                 bias=0.0, scale=0.125)
                            if prev_pr is not None:
                                pkc, ppr = prev_pr
                                for hh in range(2):
                                    nc.tensor.matmul(
                                        at_ps[:, hh * QTILE:(hh + 1) * QTILE],
                                        lhsT=vsb[b][:, pkc, hh, :],
                                        rhs=ppr[:, hh * QTILE:(hh + 1) * QTILE],
                                        start=(pkc == 0), stop=(pkc == KC - 1))
                            prev_pr = (kc, pr)
                        pkc, ppr = prev_pr
                        for hh in range(2):
                            nc.tensor.matmul(
                                at_ps[:, hh * QTILE:(hh + 1) * QTILE],
                                lhsT=vsb[b][:, pkc, hh, :],
                                rhs=ppr[:, hh * QTILE:(hh + 1) * QTILE],
                                start=(pkc == 0), stop=(pkc == KC - 1))
                        for hh in range(2):
                            # sums live in psum row 64; move to sbuf row 64, recip,
                            # PE-broadcast to 64 rows, then normalize.
                            aps = at_ps[:, hh * QTILE:(hh + 1) * QTILE]
                            smrow = wp.tile([65, QTILE], F32R, name="smrow", tag="smrow", bufs=1)
                            nc.vector.tensor_copy(smrow[64:65, :], aps[64:65, :])
                            ps_bc = pssc.tile([64, QTILE], F32, name="ps_bc", tag="pssc")
                            nc.tensor.matmul(
                                ps_bc[:],
                                lhsT=ones_sb[64:65, :],
                                rhs=smrow[64:65, :],
                                start=True, stop=True)
                            rbc = wp.tile([64, QTILE], F32, name="rbc", tag="rbc", bufs=1)
                            nc.vector.tensor_copy(rbc[:], ps_bc[:])
                            rbcr = wp.tile([64, QTILE], F32, name="rbcr", tag="rbcr", bufs=1)
                            nc.vector.reciprocal_approx_fast(out=rbcr[:], in_=rbc[:])
                            nc.vector.tensor_mul(
                                atn[b][hh][:, qt * QTILE:(qt + 1) * QTILE],
                                aps[0:64, :], rbcr[:])

                # a2a input staging
                for d in range(NCORES):
                    bb, sh = d // 4, d % 4
                    nc.sync.dma_start(a2a_in[d, 0:64, :],
                                      atn[bb][0][:, sh * 1024:(sh + 1) * 1024])
                    nc.sync.dma_start(a2a_in[d, 64:128, :],
                                      atn[bb][1][:, sh * 1024:(sh + 1) * 1024])

            nc.gpsimd.collective_compute(
                "AllToAll", ALU.bypass,
                ins=[a2a_in[:].opt()], outs=[a2a_out[:].opt()],
                replica_groups=[list(range(NCORES))])

            # ---------------- Stage C: output projection ----------------------
            with tc.tile_pool(name="cstage", bufs=1) as cp, \
                 tc.tile_pool(name="cwork", bufs=2) as cw, \
                 tc.tile_pool(name="psC", bufs=2, space="PSUM") as psC:
                atf = cp.tile([128, 8, SHARD], BF16)
                nc.sync.dma_start(atf[:], a2a_out[:].transpose([1, 0, 2]))
                for ttk in range(SHARD // 128):
                    ostage = cw.tile([128, C], F32, name="ostage", tag="ostage")
                    for half in range(2):
                        ps_o = psC.tile([128, 512], F32, name="ps_o", tag="psC")
                        for cc in range(8):
                            nc.tensor.matmul(
                                ps_o[:],
                                lhsT=atf[:, cc, ttk * 128:(ttk + 1) * 128],
                                rhs=wout_sb[:, cc, half * 512:(half + 1) * 512],
                                start=(cc == 0), stop=(cc == 7))
                        nc.vector.tensor_copy(ostage[:, half * 512:(half + 1) * 512], ps_o[:])
                    nc.sync.dma_start(out_d.ap()[ttk * 128:(ttk + 1) * 128, :], ostage[:])

    nc.compile()
    return nc


def _fold_sin(sin, g):
    out = np.empty_like(sin)
    out[:, :32] = -sin[:, :32] * g[32:]
    out[:, 32:] = sin[:, 32:] * g[:32]
    return out


def kernel(hidden_states, cos, sin, Wqkv, Wout, gq, gk):
    global _LAST_RESULT
    _install_profile_shim()

    hidden_states = np.asarray(hidden_states, dtype=np.float32)
    cos = np.asarray(cos, dtype=np.float32)
    sin = np.asarray(sin, dtype=np.float32)
    Wqkv = np.asarray(Wqkv, dtype=np.float32)
    Wout = np.asarray(Wout, dtype=np.float32)
    gq = np.asarray(gq, dtype=np.float32)
    gk = np.asarray(gk, dtype=np.float32)

    if "nc" not in _CACHE:
        _CACHE["nc"] = _build_graph()
    nc = _CACHE["nc"]

    hsT = np.ascontiguousarray(hidden_states.reshape(TOK, C).T)
    cosq = np.ascontiguousarray(cos * gq[None, :])
    sinq = _fold_sin(sin, gq)
    cosk = np.ascontiguousarray(cos * gk[None, :])
    sink = _fold_sin(sin, gk)

    in_maps = []
    for c in range(NCORES):
        wq = Wqkv[:, c * 128:(c + 1) * 128]
        wk = Wqkv[:, C + c * 128:C + (c + 1) * 128]
        wv = Wqkv[:, 2 * C + c * 128:2 * C + (c + 1) * 128]
        wqkv_loc = np.ascontiguousarray(np.concatenate([wq, wk, wv], axis=1))
        in_maps.append({
            "hsT": hsT, "wqkv": wqkv_loc, "cosq": cosq, "sinq": sinq,
            "cosk": cosk, "sink": sink, "wout": Wout,
        })

    trace = bool(os.environ.get("BASS_TRACE"))
    res = run_bass_kernel_spmd(nc, in_maps, core_ids=list(range(NCORES)), trace=trace)
    _LAST_RESULT = res

    out = np.concatenate([res.results[c]["out"] for c in range(NCORES)], axis=0)
    return out.reshape(B, N, C)


# revision 22
# speedup vs baseline: 1.8656x; 1.0267x over previous
"""Trainium2 8-core fused attention kernel (QKV proj + RMSNorm + RoPE + SDPA + out proj).

Sharding: tensor-parallel over heads. Each of the 8 cores computes 2 of the 16
heads end-to-end (QKV projection with its Wqkv column shard, per-head RMSNorm +
RoPE, full softmax attention), then an AllToAll redistributes the per-head
attention outputs so every core holds all 1024 attention channels for 1/8 of
the tokens and applies the full Wout to its token shard.

Self-contained: hardcodes all shapes from the problem spec.
"""
import os
import sys
import types

import numpy as np

sys.path.insert(0, "/opt/trn_rl_repo")

from concourse import bass, bacc, tile, mybir  # noqa: E402
from concourse.bass_utils import run_bass_kernel_spmd  # noqa: E402
from concourse.masks import make_identity  # noqa: E402

B, N, C, H, D = 2, 4096, 1024, 16, 64
NCORES = 8
TOK = B * N            # 8192 global tokens
NB = N // 128          # 32 token tiles per batch
NMACRO = N // 256      # 16 macro tiles (256 tok) per batch
QTILE = 512
NQT = N // QTILE       # 8 q tiles per batch
KC = N // 128          # 32 key chunks per batch
SHARD = TOK // NCORES  # 1024 tokens per core shard
EPS = 1e-6

F32 = mybir.dt.float32
F32R = mybir.dt.float32r
BF16 = mybir.dt.bfloat16
ALU = mybir.AluOpType
ACTF = mybir.ActivationFunctionType

_CACHE = {}
_LAST_RESULT = None


def _install_profile_shim():
    """trn_boot skips the NTFF hook when antenv.axon_hooks is missing; supply it."""
    try:
        import antenv
        if getattr(antenv, "axon_hooks", None) is not None:
            return
        from trn_agent_boot.trn_boot import _ntff_profile_via_ctypes
        hook = _ntff_profile_via_ctypes("/opt/axon/libaxon_pjrt.so")
        if hook is None:
            return
        mod = types.ModuleType("antenv.axon_hooks")
        state = {"hook": hook}
        mod.get_axon_ntff_profile_hook = lambda: state["hook"]
        mod.set_axon_ntff_profile_hook = lambda h: state.__setitem__("hook", h)
        sys.modules["antenv.axon_hooks"] = mod
        antenv.axon_hooks = mod
    except Exception:
        pass


def _build_graph():
    nc = bacc.Bacc("TRN2", target_bir_lowering=False, debug=False,
                   enable_asserts=True, num_devices=NCORES)

    hsT_d = nc.dram_tensor("hsT", [C, TOK], F32, kind="ExternalInput")
    wqkv_d = nc.dram_tensor("wqkv", [C, 384], F32, kind="ExternalInput")
    cosq_d = nc.dram_tensor("cosq", [N, D], F32, kind="ExternalInput")
    sinq_d = nc.dram_tensor("sinq", [N, D], F32, kind="ExternalInput")
    cosk_d = nc.dram_tensor("cosk", [N, D], F32, kind="ExternalInput")
    sink_d = nc.dram_tensor("sink", [N, D], F32, kind="ExternalInput")
    wout_d = nc.dram_tensor("wout", [C, C], F32, kind="ExternalInput")
    out_d = nc.dram_tensor("out", [SHARD, C], F32, kind="ExternalOutput")

    with tile.TileContext(nc) as tc:
        with tc.tile_pool(name="const", bufs=1) as constp, \
             tc.tile_pool(name="dram", bufs=1, space="DRAM") as dram:
            # resident weights
            wqkv_sb = constp.tile([128, 8, 384], F32R)
            wout_sb = constp.tile([128, 8, C], BF16)
            with tc.tile_pool(name="wtmp", bufs=2) as wtmp:
                for cc in range(8):
                    wqt = wtmp.tile([128, 384], F32, name="wqt", tag="wqt")
                    nc.sync.dma_start(
                        wqt[:], wqkv_d.ap()[cc * 128:(cc + 1) * 128, :])
                    nc.vector.tensor_copy(wqkv_sb[:, cc, :], wqt[:])
                    wtt = wtmp.tile([128, C], F32, name="wtt", tag="wtt")
                    nc.sync.dma_start(
                        wtt[:], wout_d.ap()[cc * 128:(cc + 1) * 128, :])
                    nc.vector.tensor_copy(wout_sb[:, cc, :], wtt[:])
            ident = constp.tile([128, 128], F32)
            make_identity(nc, ident[:])
            ones_f = constp.tile([65, 64], F32)
            nc.vector.memset(ones_f[:], 1.0)
            ones_sb = constp.tile([65, 64], F32R)
            nc.vector.tensor_copy(ones_sb[:], ones_f[:])
            eps_sb = constp.tile([128, 1], F32)
            nc.vector.memset(eps_sb[:], EPS)
            neg3_sb = constp.tile([128, 1], F32)
            nc.vector.memset(neg3_sb[:], -3.0)

            a2a_in = [dram.tile([NCORES, 128, SHARD // 2], BF16,
                                  name=f"a2a_in{h}", tag=f"a2a_in{h}") for h in range(2)]
            a2a_out = [dram.tile([NCORES, 128, SHARD // 2], BF16,
                                   name=f"a2a_out{h}", tag=f"a2a_out{h}") for h in range(2)]

            with tc.tile_pool(name="batch", bufs=1) as bp, \
                 tc.tile_pool(name="stream", bufs=2) as sp, \
                 tc.tile_pool(name="work", bufs=3) as wp, \
                 tc.tile_pool(name="probsp", bufs=4) as pp, \
                 tc.tile_pool(name="ps1", bufs=2, space="PSUM") as ps1, \
                 tc.tile_pool(name="pssc", bufs=2, space="PSUM") as pssc, \
                 tc.tile_pool(name="psat", bufs=1, space="PSUM") as psat:

                qT = [bp.tile([128, N], F32R, name=f"qT{b}", tag=f"qT{b}") for b in range(B)]
                kT = [bp.tile([128, N], F32R, name=f"kT{b}", tag=f"kT{b}") for b in range(B)]
                vsb = [bp.tile([128, NB, 2, 65], BF16, name=f"v{b}", tag=f"v{b}")
                       for b in range(B)]
                atn = [[bp.tile([64, N], BF16, name=f"at{b}{h}", tag=f"at{b}{h}")
                        for h in range(2)] for b in range(B)]
                for b in range(B):
                    nc.vector.memset(vsb[b][:, :, :, 64:65], 1.0)

                # ---------------- Stage A: QKV + RMSNorm + RoPE + transposes ----
                for b in range(B):
                    for mt in range(NMACRO):
                        hs_t = []
                        for cc in range(8):
                            tf_ = sp.tile([128, 256], F32, name=f"hsf{cc}", tag=f"hsf{cc}")
                            nc.sync.dma_start(
                                tf_[:], hsT_d.ap()[cc * 128:(cc + 1) * 128,
                                                   b * N + mt * 256: b * N + (mt + 1) * 256])
                            t = sp.tile([128, 256], F32R, name=f"hs{cc}", tag=f"hs{cc}")
                            nc.vector.tensor_copy(t[:], tf_[:])
                            hs_t.append(t)
                        trigC = sp.tile([128, 2, 256], F32, name="trigC", tag="trigC")
                        trigS = sp.tile([128, 2, 256], F32, name="trigS", tag="trigS")
                        for dst, dt_, off in ((trigC, cosq_d, 0), (trigC, cosk_d, 128),
                                              (trigS, sinq_d, 0), (trigS, sink_d, 128)):
                            for rep in range(2):
                                nc.sync.dma_start(
                                    dst[:, :, off + rep * D:off + (rep + 1) * D],
                                    dt_.ap()[mt * 256:(mt + 1) * 256, :]
                                    .rearrange("(s p) d -> p s d", p=128))

                        for sub in range(2):
                            tt = mt * 2 + sub  # token tile index within batch
                            ps_qkv = ps1.tile([128, 384], F32, name="ps_qkv", tag="ps1")
                            for cc in range(8):
                                nc.tensor.matmul(
                                    ps_qkv[:],
                                    lhsT=hs_t[cc][:, sub * 128:(sub + 1) * 128],
                                    rhs=wqkv_sb[:, cc, :],
                                    start=(cc == 0), stop=(cc == 7))

                            # q/k block to SBUF; all norm math on DVE (keep ACT exp-only)
                            qk_sb = wp.tile([128, 256], F32, name="qk_sb", tag="qk_sb", bufs=2)
                            nc.vector.tensor_copy(qk_sb[:], ps_qkv[:, 0:256])
                            nc.vector.tensor_copy(
                                vsb[b][:, tt, :, 0:64],
                                ps_qkv[:, 256:384].rearrange("p (h d) -> p h d", h=2))
                            # sumsq for (q h0, q h1, k h0, k h1) -> [128, 4]
                            sq = wp.tile([128, 256], F32, name="sq", tag="sq", bufs=2)
                            ssq4 = wp.tile([128, 4], F32, name="ssq4", tag="ssq4")
                            nc.vector.tensor_mul(sq[:], qk_sb[:], qk_sb[:])
                            nc.vector.tensor_reduce(
                                ssq4[:], sq[:].rearrange("p (a e) -> p a e", a=4),
                                axis=mybir.AxisListType.X, op=ALU.add)
                            # rinv = 1/sqrt(ssq/64 + eps): bit-trick seed + 1 Newton step
                            xm = wp.tile([128, 4], F32, name="xm", tag="xm")
                            nc.vector.tensor_scalar(out=xm[:], in0=ssq4[:],
                                                    scalar1=1.0 / D, scalar2=EPS,
                                                    op0=ALU.mult, op1=ALU.add)
                            yv = wp.tile([128, 4], F32, name="yv", tag="yv")
                            with nc.allow_low_precision(reason="rsqrt newton seed"):
                                nc.vector.tensor_scalar(
                                    out=yv[:].bitcast(mybir.dt.int32),
                                    in0=xm[:].bitcast(mybir.dt.int32),
                                    scalar1=1, scalar2=None, op0=ALU.arith_shift_right)
                                nc.vector.tensor_scalar(
                                    out=yv[:].bitcast(mybir.dt.int32),
                                    in0=yv[:].bitcast(mybir.dt.int32),
                                    scalar1=-1, scalar2=0x5F3759DF,
                                    op0=ALU.mult, op1=ALU.add)
                            tn = wp.tile([128, 4], F32, name="tn", tag="tn")
                            nc.vector.tensor_mul(tn[:], yv[:], yv[:])
                            nc.vector.tensor_mul(tn[:], tn[:], xm[:])
                            nc.vector.tensor_scalar(out=tn[:], in0=tn[:],
                                                    scalar1=-0.5, scalar2=1.5,
                                                    op0=ALU.mult, op1=ALU.add)
                            nc.vector.tensor_mul(yv[:], yv[:], tn[:])
                            # normalize all 4 groups, then wide rope over [128, 256]
                            qn2 = wp.tile([128, 256], F32, name="qn2", tag="qn2", bufs=2)
                            for g in range(4):
                                nc.vector.tensor_scalar(
                                    out=qn2[:, g * 64:(g + 1) * 64],
                                    in0=qk_sb[:, g * 64:(g + 1) * 64],
                                    scalar1=yv[:, g:g + 1], scalar2=None, op0=ALU.mult)
                            d_qk = wp.tile([128, 256], F32, name="d_qk", tag="d_qk", bufs=2)
                            nc.vector.tensor_mul(d_qk[:], qn2[:], trigC[:, sub, :])
                            trot = wp.tile([128, 256], F32, name="trot", tag="trot", bufs=2)
                            v4 = qn2[:].rearrange("p (a e) -> p a e", a=8)
                            s4 = trigS[:, sub, :].rearrange("p (a e) -> p a e", a=8)
                            t4 = trot[:].rearrange("p (a e) -> p a e", a=8)
                            nc.vector.tensor_mul(t4[:, 0:8:2, :], v4[:, 1:8:2, :],
                                                 s4[:, 0:8:2, :])
                            nc.vector.tensor_mul(t4[:, 1:8:2, :], v4[:, 0:8:2, :],
                                                 s4[:, 1:8:2, :])
                            nc.vector.tensor_add(d_qk[:], d_qk[:], trot[:])
                            for half, dstname in ((0, "q"), (1, "k")):
                                ps_t = ps1.tile([128, 128], F32, name="ps_t", tag="ps1")
                                nc.tensor.transpose(
                                    ps_t[:], d_qk[:, half * 128:(half + 1) * 128], ident[:])
                                dst = qT[b] if dstname == "q" else kT[b]
                                nc.vector.tensor_copy(dst[:, tt * 128:(tt + 1) * 128], ps_t[:])

                # ---------------- Stage B: attention --------------------------
                # software pipeline: emit scores(kc)+exp(kc), then attn(kc-1) so
                # the PE never waits on the ACT exp of the current chunk.
                for b in range(B):
                    for qt in range(NQT):
                        at_ps = psat.tile([65, 2 * QTILE], F32, name="at_ps", tag="psat",
                                          bufs=1)
                        prev_pr = None
                        for kc in range(KC):
                            ps_s = pssc.tile([128, 2 * QTILE], F32, name="ps_s", tag="pssc")
                            for hh in range(2):
                                nc.tensor.matmul(
                                    ps_s[:, hh * QTILE:(hh + 1) * QTILE],
                                    lhsT=kT[b][64 * hh:64 * (hh + 1),
                                               kc * 128:(kc + 1) * 128],
                                    rhs=qT[b][64 * hh:64 * (hh + 1),
                                              qt * QTILE:(qt + 1) * QTILE],
                                    start=True, stop=True)
                            pr = pp.tile([128, 2 * QTILE], BF16, name="pr", tag="pr",
                                         bufs=3)
                            nc.scalar.activation(pr[:], ps_s[:], ACTF.Exp,
                                                 bias=0.0, scale=0.125)
                            if prev_pr is not None:
                                pkc, ppr = prev_pr
                                for hh in range(2):
                                    nc.tensor.matmul(
                                        at_ps[:, hh * QTILE:(hh + 1) * QTILE],
                                        lhsT=vsb[b][:, pkc, hh, :],
                                        rhs=ppr[:, hh * QTILE:(hh + 1) * QTILE],
                                        start=(pkc == 0), stop=(pkc == KC - 1))
                            prev_pr = (kc, pr)
                        pkc, ppr = prev_pr
                        for hh in range(2):
                            nc.tensor.matmul(
                                at_ps[:, hh * QTILE:(hh + 1) * QTILE],
                                lhsT=vsb[b][:, pkc, hh, :],
                                rhs=ppr[:, hh * QTILE:(hh + 1) * QTILE],
                                start=(pkc == 0), stop=(pkc == KC - 1))
                        # evacuate attn psum to sbuf (frees psat for the next q-tile),
                        # then normalize from the sbuf copy off the critical path.
                        at_sb = wp.tile([65, 2 * QTILE], F32R, name="at_sb", tag="at_sb",
                                        bufs=1)
                        nc.vector.tensor_copy(at_sb[:], at_ps[:])
                        for hh in range(2):
                            aps = at_sb[:, hh * QTILE:(hh + 1) * QTILE]
                            ps_bc = pssc.tile([64, QTILE], F32, name="ps_bc", tag="pssc")
                            nc.tensor.matmul(
                                ps_bc[:],
                                lhsT=ones_sb[64:65, :],
                                rhs=aps[64:65, :],
                                start=True, stop=True)
                            rbc = wp.tile([64, QTILE], F32, name="rbc", tag="rbc", bufs=1)
                            nc.vector.tensor_copy(rbc[:], ps_bc[:])
                            rbcr = wp.tile([64, QTILE], F32, name="rbcr", tag="rbcr", bufs=1)
                            nc.vector.reciprocal_approx_fast(out=rbcr[:], in_=rbc[:])
                            nc.vector.tensor_mul(
                                atn[b][hh][:, qt * QTILE:(qt + 1) * QTILE],
                                aps[0:64, :], rbcr[:])

                # a2a input staging: split the shard-token dim in two so the second
                # collective overlaps the first half's output projection.
                for ha in range(2):
                    for d in range(NCORES):
                        bb, sh = d // 4, d % 4
                        base = sh * 1024 + ha * 512
                        nc.sync.dma_start(a2a_in[ha][d, 0:64, :],
                                          atn[bb][0][:, base:base + 512])
                        nc.sync.dma_start(a2a_in[ha][d, 64:128, :],
                                          atn[bb][1][:, base:base + 512])

            for ha in range(2):
                nc.gpsimd.collective_compute(
                    "AllToAll", ALU.bypass,
                    ins=[a2a_in[ha][:].opt()], outs=[a2a_out[ha][:].opt()],
                    replica_groups=[list(range(NCORES))])

            # ---------------- Stage C: output projection ----------------------
            with tc.tile_pool(name="cstage", bufs=1) as cp, \
                 tc.tile_pool(name="cwork", bufs=2) as cw, \
                 tc.tile_pool(name="psC", bufs=2, space="PSUM") as psC:
                atf = cp.tile([128, 8, SHARD], BF16)
                for ha in range(2):
                    nc.sync.dma_start(atf[:, :, ha * 512:(ha + 1) * 512],
                                      a2a_out[ha][:].transpose([1, 0, 2]))
                for ttk in range(SHARD // 128):
                    ostage = cw.tile([128, C], F32, name="ostage", tag="ostage")
                    for half in range(2):
                        ps_o = psC.tile([128, 512], F32, name="ps_o", tag="psC")
                        for cc in range(8):
                            nc.tensor.matmul(
                                ps_o[:],
                                lhsT=atf[:, cc, ttk * 128:(ttk + 1) * 128],
                                rhs=wout_sb[:, cc, half * 512:(half + 1) * 512],
                                start=(cc == 0), stop=(cc == 7))
                        nc.vector.tensor_copy(ostage[:, half * 512:(half + 1) * 512], ps_o[:])
                    nc.sync.dma_start(out_d.ap()[ttk * 128:(ttk + 1) * 128, :], ostage[:])

    nc.compile()
    return nc


def _fold_sin(sin, g):
    out = np.empty_like(sin)
    out[:, :32] = -sin[:, :32] * g[32:]
    out[:, 32:] = sin[:, 32:] * g[:32]
    return out


def kernel(hidden_states, cos, sin, Wqkv, Wout, gq, gk):
    global _LAST_RESULT
    _install_profile_shim()

    hidden_states = np.asarray(hidden_states, dtype=np.float32)
    cos = np.asarray(cos, dtype=np.float32)
    sin = np.asarray(sin, dtype=np.float32)
    Wqkv = np.asarray(Wqkv, dtype=np.float32)
    Wout = np.asarray(Wout, dtype=np.float32)
    gq = np.asarray(gq, dtype=np.float32)
    gk = np.asarray(gk, dtype=np.float32)

    if "nc" not in _CACHE:
        _CACHE["nc"] = _build_graph()
    nc = _CACHE["nc"]

    hsT = np.ascontiguousarray(hidden_states.reshape(TOK, C).T)
    cosq = np.ascontiguousarray(cos * gq[None, :])
    sinq = _fold_sin(sin, gq)
    cosk = np.ascontiguousarray(cos * gk[None, :])
    sink = _fold_sin(sin, gk)

    in_maps = []
    for c in range(NCORES):
        wq = Wqkv[:, c * 128:(c + 1) * 128]
        wk = Wqkv[:, C + c * 128:C + (c + 1) * 128]
        wv = Wqkv[:, 2 * C + c * 128:2 * C + (c + 1) * 128]
        wqkv_loc = np.ascontiguousarray(np.concatenate([wq, wk, wv], axis=1))
        in_maps.append({
            "hsT": hsT, "wqkv": wqkv_loc, "cosq": cosq, "sinq": sinq,
            "cosk": cosk, "sink": sink, "wout": Wout,
        })

    trace = bool(os.environ.get("BASS_TRACE"))
    res = run_bass_kernel_spmd(nc, in_maps, core_ids=list(range(NCORES)), trace=trace)
    _LAST_RESULT = res

    out = np.concatenate([res.results[c]["out"] for c in range(NCORES)], axis=0)
    return out.reshape(B, N, C)
